# revision 1
# baseline (speedup 1.0000x reference)
"""Bass/Trainium2 kernel for nn_DisentangleLayer (FactorGCN-style GNN layer).

Math (per reference):
  h    = x @ W_lin + b_lin                    [N, 256]
  h_em = x @ emb_w + emb_b                    [N, 64]
  s_src = h @ a_src.T ; s_dst = h @ a_dst.T   [N, 4]    (att_w = [a_src | a_dst])
  e    = sigmoid(s_src[src] + s_dst[dst] + att_b)       [E, 4]
  ev   = exp(e - max(e))     (max subtraction cancels in the normalized
                              ratio below; sigmoid output is bounded so no
                              overflow risk -> we skip it)
  denom = segsum_src(ev)                       [N, 4]
  out[n, 64l:64l+64] = segsum_src(ev_l * h_em[dst]) / denom[n, l]

Strategy:
  * Host shards edges by src-range across 8 cores (each core owns 12500
    nodes' outputs; no cross-core reduction needed).
  * Per core, edges are sorted by src and mapped to dense ranks; ranks are
    grouped into 128-node windows; each window's edge list is padded to a
    fixed number of 128-edge tiles (uniform across cores -> one compiled
    program).
  * Device: phase A computes a packed per-node table
    [h_em(64) | s_dst+att_b(4) | s_src(4) | pad] (f32, 512B rows) for all
    N nodes (replicated per core).  Edge phase gathers table rows by dst
    via indirect DMA, forms per-tile one-hot matrices S (edges x ranks)
    and S^T on-chip, and uses TensorE matmuls for the per-window segment
    sums of [ev*h_em | ev]; normalization by denom happens per window.
"""

import math
import numpy as np
from contextlib import ExitStack

P = 128
CORES = 8
IN_F = 256
D_EM = 64
L = 4

_PATCHED = False


def _apply_tile_patch():
    """walrus in this env rejects >1 sem-wait on one instruction; split the
    TileContext exit-drain waits across single-wait nops."""
    global _PATCHED
    if _PATCHED:
        return
    _PATCHED = True
    import concourse.tile as tile_mod
    import concourse.mybir as mybir
    from concourse.vector_clock import ScopedClock

    def _drain_and_barrier(self, tick_clock, wait_clock):
        nop = self.nc.sync.nop()
        wait_clock.add_sem_waits(nop.ins, ScopedClock({None: tick_clock.global_clock}))
        si = nop.ins.sync_info
        waits = list(si.on_wait) if si is not None else []
        if len(waits) > 1:
            si.on_wait = waits[:1]
            nop.ins.sync_info = si
            for i in range(1, len(waits)):
                extra = self.nc.sync.nop()
                extra.ins.sync_info = mybir.SyncInfo(
                    on_wait=waits[i : i + 1], on_update=[]
                )
        self.nc.sync.drain()
        self.nc.all_engine_barrier()
        assert self.sems is not None
        popped = self.nc._tile_sem_poison_stack.pop()
        assert popped is self._sem_poison
        self.nc.clear_and_free_semaphores(list(self.sems.allocated().values()))
        self.nc.all_engine_barrier()

    tile_mod.TileContext._drain_and_barrier = _drain_and_barrier


# ----------------------------------------------------------------------------
# host-side sharding / stream building
# ----------------------------------------------------------------------------

def _host_prep(src, dst, n_nodes, n_cores):
    """Shard edges by src range, sort by src, build per-core device streams.

    Returns (cfg, per_core) where per_core[c] is a dict of numpy arrays and
    cfg holds the uniform shape parameters.
    """
    NV = n_nodes // n_cores
    NW = (NV + P - 1) // P
    src = np.asarray(src)
    dst = np.asarray(dst)

    cores = []
    for c in range(n_cores):
        lo = c * NV
        sel = (src >= lo) & (src < lo + NV)
        es = src[sel].astype(np.int64) - lo
        ed = dst[sel].astype(np.int64)
        order = np.argsort(es, kind="stable")
        es = es[order]
        ed = ed[order]
        u, counts = np.unique(es, return_counts=True)
        K = len(u)
        ranks = np.repeat(np.arange(K, dtype=np.int64), counts)
        w = ranks // P
        cnt_w = np.bincount(w, minlength=NW)
        cores.append((ed, u, K, ranks, w, cnt_w))

    T_w = 1
    for (_, _, _, _, _, cnt_w) in cores:
        T_w = max(T_w, int(math.ceil(cnt_w.max() / P)))

    per_core = []
    for c in range(n_cores):
        ed, u, K, ranks, w, cnt_w = cores[c]
        lo = c * NV
        nslot = T_w * P
        slot_rank = np.full((NW, nslot), -1.0, np.float32)
        slot_dst = np.zeros((NW, nslot), np.int32)
        offs = np.concatenate([[0], np.cumsum(cnt_w)])
        pos = np.arange(len(ed)) - offs[w]
        slot_rank[w, pos] = (ranks % P).astype(np.float32)
        slot_dst[w, pos] = ed.astype(np.int32)

        # device layouts: [128, NW*T_w] with slot (w, i, p) -> [p, w*T_w + i]
        rank_col = (
            slot_rank.reshape(NW, T_w, P).transpose(2, 0, 1).reshape(P, NW * T_w)
        )
        didx = slot_dst.reshape(NW, T_w, P).transpose(2, 0, 1).reshape(P, NW * T_w)

        # ST one-hot bytes: stb[n, (w, i, e)] == 1 iff rank of slot
        # (w, i, e) == n.   (e is the partition index of the edge.)
        stb = np.zeros((P, NW, T_w, P), np.uint8)
        sr = slot_rank.reshape(NW, T_w, P)
        wv, iv, evi = np.nonzero(sr >= 0)
        nv = sr[wv, iv, evi].astype(np.int64)
        stb[nv, wv, iv, evi] = 1
        stb = stb.reshape(P, NW * T_w * P)

        u_pad = np.zeros(NW * P, np.int32)
        u_pad[:K] = (u + lo).astype(np.int32)
        uidx = u_pad.reshape(NW, P).T.copy()  # [128, NW]

        per_core.append(
            dict(didx=didx, rankc=rank_col, stb=stb, uidx=uidx, u=u, K=K)
        )

    cfg = dict(NV=NV, NW=NW, T_w=T_w)
    return cfg, per_core


# ----------------------------------------------------------------------------
# device program
# ----------------------------------------------------------------------------

def _split_multi_waits(nc):
    """This env's walrus accepts at most ONE sync-wait command per
    instruction.  Move extra waits onto single-wait nops inserted just
    before the instruction on the same engine (same sequencer => identical
    semantics)."""
    import concourse.mybir as mybir

    cnt = 0
    for f in nc.m.functions:
        for blk in f.blocks:
            insts = blk.instructions
            out = []
            changed = False
            for ins in insts:
                si = ins.sync_info
                waits = list(si.on_wait) if si is not None else []
                if len(waits) > 1:
                    changed = True
                    for w in waits[:-1]:
                        cnt += 1
                        nop = mybir.InstNoOp(
                            name=f"wsplit_{cnt}", ins=[], outs=[]
                        )
                        nop.engine = ins.engine
                        nop.sync_info = mybir.SyncInfo(on_wait=[w], on_update=[])
                        out.append(nop)
                    si.on_wait = waits[-1:]
                    ins.sync_info = si
                out.append(ins)
            if changed:
                blk.instructions = out
    return cnt


def _build_nc(N, NW, T_w, TC=16, split_waits=True):
    _apply_tile_patch()
    import concourse.bass as bass
    import concourse.mybir as mybir
    import concourse.tile as tile
    from concourse.masks import make_identity

    f32 = mybir.dt.float32
    i32 = mybir.dt.int32
    u8 = mybir.dt.uint8
    AF = mybir.ActivationFunctionType
    OP = mybir.AluOpType
    IOOA = bass.IndirectOffsetOnAxis

    nc = bass.Bass()
    x_d = nc.declare_dram_parameter("x", [N, IN_F], f32, isOutput=False)
    wl_d = nc.declare_dram_parameter("wl", [IN_F, IN_F], f32, isOutput=False)
    aw_d = nc.declare_dram_parameter("aw", [L, 2 * IN_F], f32, isOutput=False)
    ew_d = nc.declare_dram_parameter("ew", [IN_F, D_EM], f32, isOutput=False)
    bl_d = nc.declare_dram_parameter("bl", [IN_F, 1], f32, isOutput=False)
    embb_d = nc.declare_dram_parameter("embb", [1, D_EM], f32, isOutput=False)
    attb_d = nc.declare_dram_parameter("attb", [1, L], f32, isOutput=False)
    didx_d = nc.declare_dram_parameter("didx", [P, NW * T_w], i32, isOutput=False)
    rankc_d = nc.declare_dram_parameter("rankc", [P, NW * T_w], f32, isOutput=False)
    stb_d = nc.declare_dram_parameter("stb", [P, NW * T_w * P], u8, isOutput=False)
    uidx_d = nc.declare_dram_parameter("uidx", [P, NW], i32, isOutput=False)
    iota_d = nc.declare_dram_parameter("iota_mat", [P, P], f32, isOutput=False)
    descale_d = nc.declare_dram_parameter("descale", [P, 1], f32, isOutput=False)
    out_d = nc.declare_dram_parameter("out", [NW * P, 4 * D_EM], f32, isOutput=True)

    tbl = nc.dram_tensor("tbl", [N, P], f32)  # [h_em(64)|s_dst+attb(4)|s_src(4)|0]

    ntileA = (N + P - 1) // P
    chunks = []
    k0 = 0
    while k0 < T_w:
        chunks.append((k0, min(TC, T_w - k0)))
        k0 += TC

    with ExitStack() as ctx:
        tc = ctx.enter_context(tile.TileContext(nc))
        const = ctx.enter_context(tc.tile_pool(name="const", bufs=1))

        ident = const.tile([P, P], f32)
        make_identity(nc, ident[:])
        iota = const.tile([P, P], f32)
        nc.sync.dma_start(out=iota[:], in_=iota_d[:])
        descale = const.tile([P, 1], f32)
        nc.sync.dma_start(out=descale[:], in_=descale_d[:])

        # ---- fold weights: Wp[ic] = [emb_w | W@a_dst.T | W@a_src.T | 0] ----
        WT = [[const.tile([P, P], f32, name=f"WT_{j}_{i}") for i in range(2)] for j in range(2)]
        adT = [const.tile([P, L], f32, name=f"adT_{j}") for j in range(2)]
        asT = [const.tile([P, L], f32, name=f"asT_{j}") for j in range(2)]
        blT = [const.tile([P, 1], f32, name=f"blT_{j}") for j in range(2)]
        Wp = [const.tile([P, P], f32, name=f"Wp_{i}") for i in range(2)]
        bias_row = const.tile([1, P], f32)
        ones1 = const.tile([1, P], f32)
        bias_bc = const.tile([P, P], f32)
        attb_sb = const.tile([1, L], f32)

        with (
            tc.tile_pool(name="setup_sb", bufs=2) as ssb,
            tc.tile_pool(name="setup_ps", bufs=2, space="PSUM") as sps,
        ):
            for jc in range(2):
                nc.sync.dma_start(
                    out=adT[jc][:],
                    in_=aw_d[:, IN_F + jc * P : IN_F + (jc + 1) * P].transpose([1, 0]),
                )
                nc.sync.dma_start(
                    out=asT[jc][:],
                    in_=aw_d[:, jc * P : (jc + 1) * P].transpose([1, 0]),
                )
                nc.sync.dma_start(out=blT[jc][:], in_=bl_d[jc * P : (jc + 1) * P, :])
                for ic in range(2):
                    wt = ssb.tile([P, P], f32)
                    nc.sync.dma_start(
                        out=wt[:],
                        in_=wl_d[ic * P : (ic + 1) * P, jc * P : (jc + 1) * P],
                    )
                    tp = sps.tile([P, P], f32, space="PSUM")
                    nc.tensor.transpose(out=tp[:], in_=wt[:], identity=ident[:])
                    nc.vector.tensor_copy(out=WT[jc][ic][:], in_=tp[:])

            for ic in range(2):
                nc.gpsimd.memset(Wp[ic][:], 0)
                nc.sync.dma_start(
                    out=Wp[ic][:, 0:D_EM], in_=ew_d[ic * P : (ic + 1) * P, :]
                )
                wd_ps = sps.tile([P, 2 * L], f32, space="PSUM")
                for t, rhs_t in ((0, adT), (1, asT)):
                    for jc in range(2):
                        nc.tensor.matmul(
                            out=wd_ps[:, t * L : (t + 1) * L],
                            lhsT=WT[jc][ic][:],
                            rhs=rhs_t[jc][:],
                            start=(jc == 0),
                            stop=(jc == 1),
                        )
                nc.vector.tensor_copy(
                    out=Wp[ic][:, D_EM : D_EM + 2 * L], in_=wd_ps[:]
                )

            bias_ps = sps.tile([1, 2 * L], f32, space="PSUM")
            for t, rhs_t in ((0, adT), (1, asT)):
                for jc in range(2):
                    nc.tensor.matmul(
                        out=bias_ps[:, t * L : (t + 1) * L],
                        lhsT=blT[jc][:],
                        rhs=rhs_t[jc][:],
                        start=(jc == 0),
                        stop=(jc == 1),
                    )
            nc.gpsimd.memset(bias_row[:], 0)
            nc.sync.dma_start(out=bias_row[:, 0:D_EM], in_=embb_d[:])
            nc.sync.dma_start(out=attb_sb[:], in_=attb_d[:])
            nc.vector.tensor_tensor(
                out=bias_row[:, D_EM : D_EM + L],
                in0=bias_ps[:, 0:L],
                in1=attb_sb[:],
                op=OP.add,
            )
            nc.vector.tensor_copy(
                out=bias_row[:, D_EM + L : D_EM + 2 * L], in_=bias_ps[:, L : 2 * L]
            )
            # broadcast bias_row across partitions via K=1 matmul
            nc.gpsimd.memset(ones1[:], 1.0)
            bb_ps = sps.tile([P, P], f32, space="PSUM")
            nc.tensor.matmul(
                out=bb_ps[:], lhsT=ones1[:], rhs=bias_row[:], start=True, stop=True
            )
            nc.vector.tensor_copy(out=bias_bc[:], in_=bb_ps[:])

        # ---- phase A: build tbl[N, 128] ----
        with (
            tc.tile_pool(name="xa", bufs=3) as xa,
            tc.tile_pool(name="xt", bufs=3) as xtp,
            tc.tile_pool(name="stg", bufs=3) as stg,
            tc.tile_pool(name="psT", bufs=2, space="PSUM") as psT,
            tc.tile_pool(name="psM", bufs=2, space="PSUM") as psM,
        ):
            for i in range(ntileA):
                r0 = i * P
                pp = min(P, N - r0)
                xt = xa.tile([P, IN_F], f32)
                nc.sync.dma_start(out=xt[:pp, :], in_=x_d[r0 : r0 + pp, :])
                xTs = []
                for jc in range(2):
                    tp = psT.tile([P, P], f32, space="PSUM")
                    nc.tensor.transpose(
                        out=tp[:, :pp],
                        in_=xt[:pp, jc * P : (jc + 1) * P],
                        identity=ident[:pp, :pp],
                    )
                    xT = xtp.tile([P, P], f32)
                    nc.scalar.copy(out=xT[:, :pp], in_=tp[:, :pp])
                    xTs.append(xT)
                tab_ps = psM.tile([P, P], f32, space="PSUM")
                for jc in range(2):
                    nc.tensor.matmul(
                        out=tab_ps[:pp, :],
                        lhsT=xTs[jc][:, :pp],
                        rhs=Wp[jc][:],
                        start=(jc == 0),
                        stop=(jc == 1),
                    )
                st = stg.tile([P, P], f32)
                nc.vector.tensor_tensor(
                    out=st[:pp, :], in0=tab_ps[:pp, :], in1=bias_bc[:pp, :], op=OP.add
                )
                nc.sync.dma_start(out=tbl[r0 : r0 + pp, :], in_=st[:pp, :])

        # ---- upfront: s_src gather + streams ----
        uix = const.tile([P, NW], i32)
        nc.sync.dma_start(out=uix[:], in_=uidx_d[:])
        ssrc = const.tile([P, NW, L], f32)
        for w in range(NW):
            nc.gpsimd.indirect_dma_start(
                out=ssrc[:, w, :],
                out_offset=None,
                in_=tbl[:, :],
                in_offset=IOOA(ap=uix[:, w : w + 1], axis=0),
                element_offset=D_EM + L,
            )
        didx_sb = const.tile([P, NW * T_w], i32)
        nc.sync.dma_start(out=didx_sb[:], in_=didx_d[:])
        rankc_sb = const.tile([P, NW * T_w], f32)
        nc.sync.dma_start(out=rankc_sb[:], in_=rankc_d[:])

        # ---- edge phase ----
        with (
            tc.tile_pool(name="g", bufs=3) as gpool,
            tc.tile_pool(name="stb", bufs=3) as stbp,
            tc.tile_pool(name="st", bufs=3) as stp,
            tc.tile_pool(name="s", bufs=3) as sp,
            tc.tile_pool(name="z", bufs=4) as zp,
            tc.tile_pool(name="rev", bufs=2) as revp,
            tc.tile_pool(name="onorm", bufs=2) as onp,
            tc.tile_pool(name="psZ", bufs=3, space="PSUM") as psZ,
            tc.tile_pool(name="psU", bufs=2, space="PSUM") as psU,
        ):
            for w in range(NW):
                U_ps = psU.tile([P, 4 * D_EM + L], f32, space="PSUM")
                for (k0, tcw) in chunks:
                    c0 = w * T_w + k0
                    G = gpool.tile([P, TC, P], f32)
                    for i in range(tcw):
                        nc.gpsimd.indirect_dma_start(
                            out=G[:, i, :],
                            out_offset=None,
                            in_=tbl[:, :],
                            in_offset=IOOA(
                                ap=didx_sb[:, c0 + i : c0 + i + 1], axis=0
                            ),
                        )
                    stbits = stbp.tile([P, TC, P], u8)
                    nc.sync.dma_start(
                        out=stbits[:, :tcw, :],
                        in_=stb_d[:, c0 * P : (c0 + tcw) * P],
                    )
                    ST = stp.tile([P, TC, P], f32)
                    nc.scalar.copy(out=ST[:, :tcw, :], in_=stbits[:, :tcw, :])
                    S = sp.tile([P, TC, P], f32)
                    nc.vector.tensor_tensor(
                        out=S[:, :tcw, :],
                        in0=rankc_sb[:, c0 : c0 + tcw]
                        .unsqueeze(2)
                        .to_broadcast([P, tcw, P]),
                        in1=iota[:].unsqueeze(1).to_broadcast([P, tcw, P]),
                        op=OP.is_equal,
                    )
                    se_ps = psZ.tile([P, TC, L], f32, space="PSUM")
                    for i in range(tcw):
                        nc.tensor.matmul(
                            out=se_ps[:, i, :],
                            lhsT=ST[:, i, :],
                            rhs=ssrc[:, w, :],
                            start=True,
                            stop=True,
                        )
                    zt = zp.tile([P, TC, L], f32)
                    nc.vector.tensor_tensor(
                        out=zt[:, :tcw, :],
                        in0=se_ps[:, :tcw, :],
                        in1=G[:, :tcw, D_EM : D_EM + L],
                        op=OP.add,
                    )
                    sg = zp.tile([P, TC, L], f32)
                    nc.scalar.activation(
                        out=sg[:, :tcw, :], in_=zt[:, :tcw, :], func=AF.Sigmoid
                    )
                    Rev = revp.tile([P, TC, 4 * D_EM + L], f32)
                    nc.scalar.activation(
                        out=Rev[:, :tcw, 4 * D_EM : 4 * D_EM + L],
                        in_=sg[:, :tcw, :],
                        func=AF.Exp,
                    )
                    nc.vector.tensor_tensor(
                        out=Rev[:, :tcw, 0 : 4 * D_EM].rearrange(
                            "p t (l d) -> p t l d", l=L
                        ),
                        in0=G[:, :tcw, 0:D_EM]
                        .unsqueeze(2)
                        .to_broadcast([P, tcw, L, D_EM]),
                        in1=Rev[:, :tcw, 4 * D_EM : 4 * D_EM + L]
                        .unsqueeze(3)
                        .to_broadcast([P, tcw, L, D_EM]),
                        op=OP.mult,
                    )
                    for i in range(tcw):
                        nc.tensor.matmul(
                            out=U_ps[:, :],
                            lhsT=S[:, i, :],
                            rhs=Rev[:, i, :],
                            start=(k0 == 0 and i == 0),
                            stop=(k0 + tcw == T_w and i == tcw - 1),
                        )
                dn = onp.tile([P, L], f32)
                nc.vector.tensor_scalar(
                    out=dn[:],
                    in0=U_ps[:, 4 * D_EM : 4 * D_EM + L],
                    scalar1=1e-30,
                    scalar2=None,
                    op0=OP.add,
                )
                dnr = onp.tile([P, L], f32)
                nc.vector.reciprocal(out=dnr[:], in_=dn[:])
                ot = onp.tile([P, 4 * D_EM], f32)
                nc.vector.tensor_tensor(
                    out=ot[:].rearrange("p (l d) -> p l d", l=L),
                    in0=U_ps[:, 0 : 4 * D_EM].rearrange("p (l d) -> p l d", l=L),
                    in1=dnr[:].unsqueeze(2).to_broadcast([P, L, D_EM]),
                    op=OP.mult,
                )
                nc.sync.dma_start(out=out_d[w * P : (w + 1) * P, :], in_=ot[:])

    if split_waits:
        _split_multi_waits(nc)
    return nc


# ----------------------------------------------------------------------------
# public entry point
# ----------------------------------------------------------------------------

_NC_CACHE = {}


def _get_nc(N, NW, T_w, TC=16):
    key = (N, NW, T_w, TC)
    if key not in _NC_CACHE:
        _NC_CACHE[key] = _build_nc(N, NW, T_w, TC)
    return _NC_CACHE[key]


def _make_in_maps(x, W_lin, b_lin, att_w, att_b, emb_w, emb_b, per_core, n_cores):
    x = np.ascontiguousarray(np.asarray(x, np.float32))
    shared = dict(
        x=x,
        wl=np.ascontiguousarray(np.asarray(W_lin, np.float32)),
        aw=np.ascontiguousarray(np.asarray(att_w, np.float32)),
        ew=np.ascontiguousarray(np.asarray(emb_w, np.float32)),
        bl=np.ascontiguousarray(np.asarray(b_lin, np.float32).reshape(-1, 1)),
        embb=np.ascontiguousarray(np.asarray(emb_b, np.float32).reshape(1, -1)),
        attb=np.ascontiguousarray(np.asarray(att_b, np.float32).reshape(1, -1)),
        iota_mat=np.broadcast_to(
            np.arange(P, dtype=np.float32), (P, P)
        ).copy(),
        descale=(1.0 / (1 << (np.arange(P) // 16))).astype(np.float32).reshape(P, 1),
    )
    in_maps = []
    for c in range(n_cores):
        m = dict(shared)
        m["didx"] = per_core[c]["didx"]
        m["rankc"] = per_core[c]["rankc"]
        m["stb"] = per_core[c]["stb"]
        m["uidx"] = per_core[c]["uidx"]
        in_maps.append(m)
    return in_maps


def kernel(x, src, dst, W_lin, b_lin, att_w, att_b, emb_w, emb_b):
    from concourse.bass_utils import run_bass_kernel_spmd

    x = np.asarray(x)
    N = x.shape[0]
    cfg, per_core = _host_prep(src, dst, N, CORES)
    nc = _get_nc(N, cfg["NW"], cfg["T_w"])
    in_maps = _make_in_maps(
        x, W_lin, b_lin, att_w, att_b, emb_w, emb_b, per_core, CORES
    )
    res = run_bass_kernel_spmd(nc, in_maps, list(range(CORES)))
    out = np.zeros((N, 4 * D_EM), np.float32)
    NV = cfg["NV"]
    for c in range(CORES):
        K = per_core[c]["K"]
        u = per_core[c]["u"]
        out[c * NV + u] = res.results[c]["out"][:K]
    return out



# revision 15
# speedup vs baseline: 1.1574x; 1.1574x over previous
"""Bass/Trainium2 kernel for nn_DisentangleLayer (FactorGCN-style GNN layer).

Math (per reference):
  h    = x @ W_lin + b_lin                    [N, 256]
  h_em = x @ emb_w + emb_b                    [N, 64]
  s_src = h @ a_src.T ; s_dst = h @ a_dst.T   [N, 4]    (att_w = [a_src | a_dst])
  ev   = exp(sigmoid(s_src[src] + s_dst[dst] + att_b))  [E, 4]
  denom = segsum_src(ev)                       [N, 4]
  out[n, 64l:64l+64] = segsum_src(ev_l * h_em[dst]) / denom[n, l]

Strategy (v4):
  * Host shards edges by src-range across 8 cores (each core owns NV=12500
    nodes; no cross-core reduction).  Edges are sorted by src, mapped to
    dense ranks, grouped into 128-rank windows.  Windows are grouped into
    supers of SW=7; within each window edges are split by dst bank
    (4 banks of NP/4 table rows so bank-local indices fit int16) and each
    (window, bank) run is padded to a uniform T_b tiles of 128 edge slots.
    Slot column order is (super, bank, window-in-super, tile) so one
    InstDMAGatherAnt per (super, bank) bulk-gathers 7*T_b*128 table rows.
  * Phase A builds the packed per-node table tbl[N,128] f32 =
    [h_em(64) | s_dst+att_b(4) | s_src(4) | pad] with bf16 matmuls from a
    host-transposed bf16 x; bias via a K=1 ones matmul.
  * Edge phase: S one-hot from rankc via DVE is_equal; ST = PE transpose of
    S (batched PSUM->SBUF copies on Act); per-tile se matmuls against
    per-window gathered s_src rows; ev = exp(sigmoid(se + s_dst)) on Act;
    Rev = [ev_l*h_em | ev] on DVE; per-tile bf16 matmuls U_ps[w] += S^T@Rev
    accumulate each window's segment sums in PSUM; normalize and store.
"""

import math
import numpy as np
from contextlib import ExitStack

P = 128
CORES = 8
IN_F = 256
D_EM = 64
L = 4
SW = 4      # windows per super
NB = 4      # dst banks
SUB = 14    # tiles per elementwise sub-chunk
TB_A = 8    # node tiles per phase-A block

_PATCHED = False


def _apply_tile_patch():
    """walrus in this env rejects >1 sem-wait on one instruction; split the
    TileContext exit-drain waits across single-wait nops."""
    global _PATCHED
    if _PATCHED:
        return
    _PATCHED = True
    import concourse.tile as tile_mod
    import concourse.mybir as mybir
    from concourse.vector_clock import ScopedClock

    def _drain_and_barrier(self, tick_clock, wait_clock):
        nop = self.nc.sync.nop()
        wait_clock.add_sem_waits(nop.ins, ScopedClock({None: tick_clock.global_clock}))
        si = nop.ins.sync_info
        waits = list(si.on_wait) if si is not None else []
        if len(waits) > 1:
            si.on_wait = waits[:1]
            nop.ins.sync_info = si
            for i in range(1, len(waits)):
                extra = self.nc.sync.nop()
                extra.ins.sync_info = mybir.SyncInfo(
                    on_wait=waits[i : i + 1], on_update=[]
                )
        self.nc.sync.drain()
        self.nc.all_engine_barrier()
        assert self.sems is not None
        popped = self.nc._tile_sem_poison_stack.pop()
        assert popped is self._sem_poison
        self.nc.clear_and_free_semaphores(list(self.sems.allocated().values()))
        self.nc.all_engine_barrier()

    tile_mod.TileContext._drain_and_barrier = _drain_and_barrier


# ----------------------------------------------------------------------------
# host-side sharding / stream building
# ----------------------------------------------------------------------------

def _host_prep(src, dst, n_nodes, n_cores):
    """Shard edges by src range, sort by src, build per-core device streams."""
    import ml_dtypes

    NV = n_nodes // n_cores
    NW0 = (NV + P - 1) // P
    NS = (NW0 + SW - 1) // SW         # supers
    NW = NS * SW                      # padded to whole supers (tail windows empty)
    NP_ = ((n_nodes + P - 1) // P) * P
    BANK = NP_ // NB
    src = np.asarray(src)
    dst = np.asarray(dst)

    cores = []
    T_b = 1
    for c in range(n_cores):
        lo = c * NV
        sel = (src >= lo) & (src < lo + NV)
        es = src[sel].astype(np.int64) - lo
        ed = dst[sel].astype(np.int64)
        order = np.argsort(es, kind="stable")
        es = es[order]
        ed = ed[order]
        u, counts = np.unique(es, return_counts=True)
        K = len(u)
        ranks = np.repeat(np.arange(K, dtype=np.int64), counts)
        w = ranks // P
        b = ed // BANK
        # per (window, bank) counts
        cnt_wb = np.zeros((NW, NB), np.int64)
        np.add.at(cnt_wb, (w, b), 1)
        T_b = max(T_b, int(math.ceil(cnt_wb.max() / P)))
        cores.append((ed, u, K, ranks, w, b, cnt_wb))

    T_w = NB * T_b
    NCOL = NS * NB * SW * T_b

    per_core = []
    for c in range(n_cores):
        ed, u, K, ranks, w, b, cnt_wb = cores[c]
        lo = c * NV

        # order edges by (window, bank, rank-order) and compute positions
        key = (w * NB + b)
        order2 = np.argsort(key, kind="stable")
        ed2 = ed[order2]
        w2 = w[order2]
        b2 = b[order2]
        r2 = ranks[order2]
        # position within the (w, b) group
        offs = np.zeros(NW * NB + 1, np.int64)
        offs[1:] = np.cumsum(cnt_wb.reshape(-1))
        pos = np.arange(len(ed2)) - offs[key[order2]]
        t = pos // P
        p = pos % P
        s = w2 // SW
        wi = w2 % SW
        col = ((s * NB + b2) * SW + wi) * T_b + t

        rankc = np.full((P, NCOL), -1.0, np.float32)
        rankc[p, col] = (r2 % P).astype(np.float32)

        # int16 gather index stream: flat position = col*128 + p;
        # value = bank-local row of dst
        gidx = np.zeros((16, NCOL * P // 16), np.int16)
        fpos = col * P + p
        gidx[fpos % 16, fpos // 16] = (ed2 - b2 * BANK).astype(np.int16)
        gidx = np.tile(gidx, (8, 1))  # replicate across the 8 Q7 cores

        u_pad = np.zeros(NW * P, np.int32)
        u_pad[:K] = (u + lo).astype(np.int32)
        uidx = u_pad.reshape(NW, P).T.copy()  # [128, NW]

        per_core.append(
            dict(
                gidx=gidx,
                rankc=rankc.astype(ml_dtypes.bfloat16),
                uidx=uidx,
                u=u,
                K=K,
            )
        )

    cfg = dict(NV=NV, NW=NW, T_w=T_w, T_b=T_b, NS=NS)
    return cfg, per_core


# ----------------------------------------------------------------------------
# device program
# ----------------------------------------------------------------------------

def _split_multi_waits(nc):
    """This env's walrus accepts at most ONE sync-wait command per
    instruction.  Move extra waits onto single-wait nops inserted just
    before the instruction on the same engine."""
    import concourse.mybir as mybir

    cnt = 0
    for f in nc.m.functions:
        for blk in f.blocks:
            insts = blk.instructions
            out = []
            changed = False
            for ins in insts:
                si = ins.sync_info
                waits = list(si.on_wait) if si is not None else []
                if len(waits) > 1:
                    changed = True
                    for w in waits[:-1]:
                        cnt += 1
                        nop = mybir.InstNoOp(
                            name=f"wsplit_{cnt}", ins=[], outs=[]
                        )
                        nop.engine = ins.engine
                        nop.sync_info = mybir.SyncInfo(on_wait=[w], on_update=[])
                        out.append(nop)
                    si.on_wait = waits[-1:]
                    ins.sync_info = si
                out.append(ins)
            if changed:
                blk.instructions = out
    return cnt


def _build_nc(N, NW, T_w, split_waits=True):
    _apply_tile_patch()
    import concourse.bass as bass
    import concourse.mybir as mybir
    import concourse.tile as tile
    from concourse.masks import make_identity
    from concourse import library_config
    from concourse.library_overlay import lower_extended_insts

    f32 = mybir.dt.float32
    bf16 = mybir.dt.bfloat16
    i32 = mybir.dt.int32
    i16 = mybir.dt.int16
    AF = mybir.ActivationFunctionType
    OP = mybir.AluOpType
    IOOA = bass.IndirectOffsetOnAxis

    assert T_w % NB == 0
    T_b = T_w // NB
    NS = (NW + SW - 1) // SW
    assert NS * SW == NW, (NS, NW)
    NCOL = NS * NB * SW * T_b
    RUN = SW * T_b              # tiles per (super, bank) gather run
    NP_ = ((N + P - 1) // P) * P
    BANK = NP_ // NB
    ntileA = NP_ // P

    nc = bass.Bass()
    xT_d = nc.declare_dram_parameter("xT", [IN_F, NP_], bf16, isOutput=False)
    wl_d = nc.declare_dram_parameter("wl", [IN_F, IN_F], f32, isOutput=False)
    aw_d = nc.declare_dram_parameter("aw", [L, 2 * IN_F], f32, isOutput=False)
    ew_d = nc.declare_dram_parameter("ew", [IN_F, D_EM], f32, isOutput=False)
    bl_d = nc.declare_dram_parameter("bl", [IN_F, 1], f32, isOutput=False)
    embb_d = nc.declare_dram_parameter("embb", [1, D_EM], f32, isOutput=False)
    attb_d = nc.declare_dram_parameter("attb", [1, L], f32, isOutput=False)
    gidx_d = nc.declare_dram_parameter("gidx", [P, NCOL * P // 16], i16, isOutput=False)
    rankc_d = nc.declare_dram_parameter("rankc", [P, NCOL], bf16, isOutput=False)
    uidx_d = nc.declare_dram_parameter("uidx", [P, NW], i32, isOutput=False)
    iota_d = nc.declare_dram_parameter("iota_mat", [P, P], bf16, isOutput=False)
    out_d = nc.declare_dram_parameter("out", [NW * P, 4 * D_EM], f32, isOutput=True)

    tbl = nc.dram_tensor("tbl", [NP_, P], f32)  # [h_em(64)|s_dst+attb(4)|s_src(4)|0]

    with ExitStack() as ctx:
        tc = ctx.enter_context(tile.TileContext(nc))
        nc.gpsimd.load_library(library_config.mlp)
        const = ctx.enter_context(tc.tile_pool(name="const", bufs=1))

        ident = const.tile([P, P], f32)
        make_identity(nc, ident[:])
        identb = const.tile([P, P], bf16)
        nc.vector.tensor_copy(out=identb[:], in_=ident[:])
        iotab = const.tile([P, P], bf16)
        nc.sync.dma_start(out=iotab[:], in_=iota_d[:])

        # ---- fold weights (f32): Wp[ic] = [emb_w | W@a_dst.T | W@a_src.T | 0]
        WT = [[const.tile([P, P], f32, name=f"WT_{j}_{i}") for i in range(2)] for j in range(2)]
        adT = [const.tile([P, L], f32, name=f"adT_{j}") for j in range(2)]
        asT = [const.tile([P, L], f32, name=f"asT_{j}") for j in range(2)]
        blT = [const.tile([P, 1], f32, name=f"blT_{j}") for j in range(2)]
        Wp = [const.tile([P, P], f32, name=f"Wp_{i}") for i in range(2)]
        Wpb = [const.tile([P, P], bf16, name=f"Wpb_{i}") for i in range(2)]
        bias_row = const.tile([1, P], f32)
        bias_rowb = const.tile([1, P], bf16)
        ones1b = const.tile([1, P], bf16)
        attb_sb = const.tile([1, L], f32)

        with (
            tc.tile_pool(name="setup_sb", bufs=2) as ssb,
            tc.tile_pool(name="setup_ps", bufs=2, space="PSUM") as sps,
        ):
            for jc in range(2):
                nc.sync.dma_start(
                    out=adT[jc][:],
                    in_=aw_d[:, IN_F + jc * P : IN_F + (jc + 1) * P].transpose([1, 0]),
                )
                nc.sync.dma_start(
                    out=asT[jc][:],
                    in_=aw_d[:, jc * P : (jc + 1) * P].transpose([1, 0]),
                )
                nc.sync.dma_start(out=blT[jc][:], in_=bl_d[jc * P : (jc + 1) * P, :])
                for ic in range(2):
                    wt = ssb.tile([P, P], f32)
                    nc.sync.dma_start(
                        out=wt[:],
                        in_=wl_d[ic * P : (ic + 1) * P, jc * P : (jc + 1) * P],
                    )
                    tp = sps.tile([P, P], f32, space="PSUM")
                    nc.tensor.transpose(out=tp[:], in_=wt[:], identity=ident[:])
                    nc.vector.tensor_copy(out=WT[jc][ic][:], in_=tp[:])

            for ic in range(2):
                nc.gpsimd.memset(Wp[ic][:], 0)
                nc.sync.dma_start(
                    out=Wp[ic][:, 0:D_EM], in_=ew_d[ic * P : (ic + 1) * P, :]
                )
                wd_ps = sps.tile([P, 2 * L], f32, space="PSUM")
                for t, rhs_t in ((0, adT), (1, asT)):
                    for jc in range(2):
                        nc.tensor.matmul(
                            out=wd_ps[:, t * L : (t + 1) * L],
                            lhsT=WT[jc][ic][:],
                            rhs=rhs_t[jc][:],
                            start=(jc == 0),
                            stop=(jc == 1),
                        )
                nc.vector.tensor_copy(
                    out=Wp[ic][:, D_EM : D_EM + 2 * L], in_=wd_ps[:]
                )
                nc.vector.tensor_copy(out=Wpb[ic][:], in_=Wp[ic][:])

            bias_ps = sps.tile([1, 2 * L], f32, space="PSUM")
            for t, rhs_t in ((0, adT), (1, asT)):
                for jc in range(2):
                    nc.tensor.matmul(
                        out=bias_ps[:, t * L : (t + 1) * L],
                        lhsT=blT[jc][:],
                        rhs=rhs_t[jc][:],
                        start=(jc == 0),
                        stop=(jc == 1),
                    )
            nc.gpsimd.memset(bias_row[:], 0)
            nc.sync.dma_start(out=bias_row[:, 0:D_EM], in_=embb_d[:])
            nc.sync.dma_start(out=attb_sb[:], in_=attb_d[:])
            nc.vector.tensor_tensor(
                out=bias_row[:, D_EM : D_EM + L],
                in0=bias_ps[:, 0:L],
                in1=attb_sb[:],
                op=OP.add,
            )
            nc.vector.tensor_copy(
                out=bias_row[:, D_EM + L : D_EM + 2 * L], in_=bias_ps[:, L : 2 * L]
            )
            nc.vector.tensor_copy(out=bias_rowb[:], in_=bias_row[:])
            nc.gpsimd.memset(ones1b[:], 1.0)

        # ---- phase A: tbl[N, 128] = x @ Wp + bias (bf16 matmuls, f32 out) ----
        with (
            tc.tile_pool(name="xa", bufs=2) as xap,
            tc.tile_pool(name="stgA", bufs=4) as stgA,
            tc.tile_pool(name="psM", bufs=4, space="PSUM") as psM,
        ):
            for b0 in range(0, ntileA, TB_A):
                bn = min(TB_A, ntileA - b0)
                xa0 = xap.tile([P, TB_A * P], bf16)
                xa1 = xap.tile([P, TB_A * P], bf16)
                nc.sync.dma_start(
                    out=xa0[:, : bn * P], in_=xT_d[0:P, b0 * P : (b0 + bn) * P]
                )
                nc.sync.dma_start(
                    out=xa1[:, : bn * P], in_=xT_d[P : 2 * P, b0 * P : (b0 + bn) * P]
                )
                for t in range(bn):
                    ps = psM.tile([P, P], f32, space="PSUM")
                    nc.tensor.matmul(
                        out=ps[:],
                        lhsT=xa0[:, t * P : (t + 1) * P],
                        rhs=Wpb[0][:],
                        start=True,
                        stop=False,
                    )
                    nc.tensor.matmul(
                        out=ps[:],
                        lhsT=xa1[:, t * P : (t + 1) * P],
                        rhs=Wpb[1][:],
                        start=False,
                        stop=False,
                    )
                    nc.tensor.matmul(
                        out=ps[:],
                        lhsT=ones1b[:],
                        rhs=bias_rowb[:],
                        start=False,
                        stop=True,
                    )
                    st = stgA.tile([P, P], f32)
                    nc.scalar.copy(out=st[:], in_=ps[:])
                    r0 = (b0 + t) * P
                    nc.sync.dma_start(out=tbl[r0 : r0 + P, :], in_=st[:])

        # ---- upfront: per-window s_src gather + streams ----
        uix = const.tile([P, NW], i32)
        nc.sync.dma_start(out=uix[:], in_=uidx_d[:])
        ssrcf = const.tile([P, NW, L], f32)
        for w in range(NW):
            nc.gpsimd.indirect_dma_start(
                out=ssrcf[:, w, :],
                out_offset=None,
                in_=tbl[:, :],
                in_offset=IOOA(ap=uix[:, w : w + 1], axis=0),
                element_offset=D_EM + L,
            )
        ssrcb = const.tile([P, NW, L], bf16)
        nc.vector.tensor_copy(out=ssrcb[:], in_=ssrcf[:])
        rankc_sb = const.tile([P, NCOL], bf16)
        nc.sync.dma_start(out=rankc_sb[:], in_=rankc_d[:])

        # ---- edge phase ----
        # pre-allocate gpsimd registers for gather counts (one per value)
        _qn_regs = {}
        for q0 in range(0, RUN, 8):
            qv = min(8, RUN - q0) * P
            if qv not in _qn_regs:
                _qn_regs[qv] = nc.gpsimd.to_reg(qv)
        with (
            tc.tile_pool(name="gx", bufs=2) as gxp,
            tc.tile_pool(name="g", bufs=2) as gpool,
            tc.tile_pool(name="s", bufs=2) as sp,
            tc.tile_pool(name="st", bufs=3) as stp,
            tc.tile_pool(name="z", bufs=3) as zp,
            tc.tile_pool(name="rev", bufs=3) as revp,
            tc.tile_pool(name="onorm", bufs=2) as onp,
            tc.tile_pool(name="psT", bufs=2, space="PSUM") as psT,
            tc.tile_pool(name="psS", bufs=2, space="PSUM") as psS,
            tc.tile_pool(name="psU", bufs=1, space="PSUM") as psU,
        ):
            for s in range(NS):
                gx = gxp.tile([P, NB * RUN * P // 16], i16)
                nc.sync.dma_start(
                    out=gx[:],
                    in_=gidx_d[
                        :, s * NB * RUN * P // 16 : (s + 1) * NB * RUN * P // 16
                    ],
                )
                U_s = psU.tile([P, SW, 512], f32, space="PSUM")
                for b in range(NB):
                    G = gpool.tile([P, RUN, P], f32)
                    # HW limit: <=1024 indices per InstDMAGatherAnt
                    for q0 in range(0, RUN, 8):
                        qn = min(8, RUN - q0)
                        nc.gpsimd.dma_gather(
                            G[:, q0 : q0 + qn, :],
                            tbl[b * BANK : (b + 1) * BANK, :],
                            gx[
                                :,
                                (b * RUN + q0) * 8 : (b * RUN + q0 + qn) * 8,
                            ],
                            qn * P,
                            _qn_regs[qn * P],
                            P,
                        )
                    c0 = (s * NB + b) * RUN
                    S = sp.tile([P, RUN, P], bf16)
                    nc.vector.tensor_tensor(
                        out=S[:],
                        in0=rankc_sb[:, c0 : c0 + RUN]
                        .unsqueeze(2)
                        .to_broadcast([P, RUN, P]),
                        in1=iotab[:].unsqueeze(1).to_broadcast([P, RUN, P]),
                        op=OP.is_equal,
                    )
                    for k0 in range(0, RUN, SUB):
                        kn = min(SUB, RUN - k0)
                        STb = stp.tile([P, SUB, P], bf16)
                        for g0 in range(0, kn, 4):
                            gn = min(4, kn - g0)
                            pT = psT.tile([P, 4, P], bf16, space="PSUM")
                            for j in range(gn):
                                nc.tensor.transpose(
                                    out=pT[:, j, :],
                                    in_=S[:, k0 + g0 + j, :],
                                    identity=identb[:],
                                )
                            nc.scalar.copy(
                                out=STb[:, g0 : g0 + gn, :], in_=pT[:, :gn, :]
                            )
                        se_ps = psS.tile([P, SUB, L], f32, space="PSUM")
                        for i in range(kn):
                            wi = (k0 + i) // T_b
                            nc.tensor.matmul(
                                out=se_ps[:, i, :],
                                lhsT=STb[:, i, :],
                                rhs=ssrcb[:, s * SW + wi, :],
                                start=True,
                                stop=True,
                            )
                        zt = zp.tile([P, SUB, L], f32)
                        nc.vector.tensor_tensor(
                            out=zt[:, :kn, :],
                            in0=se_ps[:, :kn, :],
                            in1=G[:, k0 : k0 + kn, D_EM : D_EM + L],
                            op=OP.add,
                        )
                        sg = zp.tile([P, SUB, L], f32)
                        nc.scalar.activation(
                            out=sg[:, :kn, :], in_=zt[:, :kn, :], func=AF.Sigmoid
                        )
                        ev = zp.tile([P, SUB, L], f32)
                        nc.scalar.activation(
                            out=ev[:, :kn, :], in_=sg[:, :kn, :], func=AF.Exp
                        )
                        Rev = revp.tile([P, SUB, 4 * D_EM + L], bf16)
                        nc.vector.tensor_tensor(
                            out=Rev[:, :kn, 0 : 4 * D_EM].rearrange(
                                "p t (l d) -> p t l d", l=L
                            ),
                            in0=G[:, k0 : k0 + kn, 0:D_EM]
                            .unsqueeze(2)
                            .to_broadcast([P, kn, L, D_EM]),
                            in1=ev[:, :kn, :]
                            .unsqueeze(3)
                            .to_broadcast([P, kn, L, D_EM]),
                            op=OP.mult,
                        )
                        nc.vector.tensor_copy(
                            out=Rev[:, :kn, 4 * D_EM : 4 * D_EM + L],
                            in_=ev[:, :kn, :],
                        )
                        for i in range(kn):
                            t = k0 + i
                            wi = t // T_b
                            tb = t % T_b
                            nc.tensor.matmul(
                                out=U_s[:, wi, 0 : 4 * D_EM + L],
                                lhsT=S[:, t, :],
                                rhs=Rev[:, i, :],
                                start=(b == 0 and tb == 0),
                                stop=(b == NB - 1 and tb == T_b - 1),
                            )
                for wi in range(SW):
                    w = s * SW + wi
                    dn = onp.tile([P, L], f32)
                    nc.vector.tensor_scalar(
                        out=dn[:],
                        in0=U_s[:, wi, 4 * D_EM : 4 * D_EM + L],
                        scalar1=1e-30,
                        scalar2=None,
                        op0=OP.add,
                    )
                    dnr = onp.tile([P, L], f32)
                    nc.vector.reciprocal(out=dnr[:], in_=dn[:])
                    ot = onp.tile([P, 4 * D_EM], f32)
                    nc.vector.tensor_tensor(
                        out=ot[:].rearrange("p (l d) -> p l d", l=L),
                        in0=U_s[:, wi, 0 : 4 * D_EM].rearrange(
                            "p (l d) -> p l d", l=L
                        ),
                        in1=dnr[:].unsqueeze(2).to_broadcast([P, L, D_EM]),
                        op=OP.mult,
                    )
                    nc.sync.dma_start(
                        out=out_d[w * P : (w + 1) * P, :], in_=ot[:]
                    )

    if split_waits:
        _split_multi_waits(nc)
    lower_extended_insts(nc)
    return nc


# ----------------------------------------------------------------------------
# public entry point
# ----------------------------------------------------------------------------

_NC_CACHE = {}


def _get_nc(N, NW, T_w):
    key = (N, NW, T_w)
    if key not in _NC_CACHE:
        _NC_CACHE[key] = _build_nc(N, NW, T_w)
    return _NC_CACHE[key]


def _make_in_maps(x, W_lin, b_lin, att_w, att_b, emb_w, emb_b, per_core, n_cores):
    import ml_dtypes

    x = np.asarray(x, np.float32)
    N = x.shape[0]
    NP_ = ((N + P - 1) // P) * P
    xT = np.zeros((IN_F, NP_), ml_dtypes.bfloat16)
    xT[:, :N] = x.T.astype(ml_dtypes.bfloat16)
    shared = dict(
        xT=np.ascontiguousarray(xT),
        wl=np.ascontiguousarray(np.asarray(W_lin, np.float32)),
        aw=np.ascontiguousarray(np.asarray(att_w, np.float32)),
        ew=np.ascontiguousarray(np.asarray(emb_w, np.float32)),
        bl=np.ascontiguousarray(np.asarray(b_lin, np.float32).reshape(-1, 1)),
        embb=np.ascontiguousarray(np.asarray(emb_b, np.float32).reshape(1, -1)),
        attb=np.ascontiguousarray(np.asarray(att_b, np.float32).reshape(1, -1)),
        iota_mat=np.ascontiguousarray(
            np.broadcast_to(np.arange(P), (P, P)).astype(ml_dtypes.bfloat16)
        ),
    )
    in_maps = []
    for c in range(n_cores):
        m = dict(shared)
        m["gidx"] = per_core[c]["gidx"]
        m["rankc"] = per_core[c]["rankc"]
        m["uidx"] = per_core[c]["uidx"]
        in_maps.append(m)
    return in_maps


def kernel(x, src, dst, W_lin, b_lin, att_w, att_b, emb_w, emb_b):
    from concourse.bass_utils import run_bass_kernel_spmd

    x = np.asarray(x)
    N = x.shape[0]
    cfg, per_core = _host_prep(src, dst, N, CORES)
    nc = _get_nc(N, cfg["NW"], cfg["T_w"])
    in_maps = _make_in_maps(
        x, W_lin, b_lin, att_w, att_b, emb_w, emb_b, per_core, CORES
    )
    res = run_bass_kernel_spmd(nc, in_maps, list(range(CORES)))
    out = np.zeros((N, 4 * D_EM), np.float32)
    NV = cfg["NV"]
    for c in range(CORES):
        K = per_core[c]["K"]
        u = per_core[c]["u"]
        out[c * NV + u] = res.results[c]["out"][:K]
    return out


# revision 19
# speedup vs baseline: 1.2457x; 1.0763x over previous
"""Bass/Trainium2 kernel for nn_DisentangleLayer (FactorGCN-style GNN layer).

Math (per reference):
  h    = x @ W_lin + b_lin                    [N, 256]
  h_em = x @ emb_w + emb_b                    [N, 64]
  s_src = h @ a_src.T ; s_dst = h @ a_dst.T   [N, 4]    (att_w = [a_src | a_dst])
  ev   = exp(sigmoid(s_src[src] + s_dst[dst] + att_b))  [E, 4]
  denom = segsum_src(ev)                       [N, 4]
  out[n, 64l:64l+64] = segsum_src(ev_l * h_em[dst]) / denom[n, l]

Strategy (v4):
  * Host shards edges by src-range across 8 cores (each core owns NV=12500
    nodes; no cross-core reduction).  Edges are sorted by src, mapped to
    dense ranks, grouped into 128-rank windows.  Windows are grouped into
    supers of SW=7; within each window edges are split by dst bank
    (4 banks of NP/4 table rows so bank-local indices fit int16) and each
    (window, bank) run is padded to a uniform T_b tiles of 128 edge slots.
    Slot column order is (super, bank, window-in-super, tile) so one
    InstDMAGatherAnt per (super, bank) bulk-gathers 7*T_b*128 table rows.
  * Phase A builds the packed per-node table tbl[N,128] f32 =
    [h_em(64) | s_dst+att_b(4) | s_src(4) | pad] with bf16 matmuls from a
    host-transposed bf16 x; bias via a K=1 ones matmul.
  * Edge phase: S one-hot from rankc via DVE is_equal; ST = PE transpose of
    S (batched PSUM->SBUF copies on Act); per-tile se matmuls against
    per-window gathered s_src rows; ev = exp(sigmoid(se + s_dst)) on Act;
    Rev = [ev_l*h_em | ev] on DVE; per-tile bf16 matmuls U_ps[w] += S^T@Rev
    accumulate each window's segment sums in PSUM; normalize and store.
"""

import math
import numpy as np
from contextlib import ExitStack

P = 128
CORES = 8
IN_F = 256
D_EM = 64
L = 4
SW = 4      # windows per super
NB = 4      # dst banks
SUB = 14    # tiles per elementwise sub-chunk
TB_A = 8    # node tiles per phase-A block

_PATCHED = False


def _apply_tile_patch():
    """walrus in this env rejects >1 sem-wait on one instruction; split the
    TileContext exit-drain waits across single-wait nops."""
    global _PATCHED
    if _PATCHED:
        return
    _PATCHED = True
    import concourse.tile as tile_mod
    import concourse.mybir as mybir
    from concourse.vector_clock import ScopedClock

    def _drain_and_barrier(self, tick_clock, wait_clock):
        nop = self.nc.sync.nop()
        wait_clock.add_sem_waits(nop.ins, ScopedClock({None: tick_clock.global_clock}))
        si = nop.ins.sync_info
        waits = list(si.on_wait) if si is not None else []
        if len(waits) > 1:
            si.on_wait = waits[:1]
            nop.ins.sync_info = si
            for i in range(1, len(waits)):
                extra = self.nc.sync.nop()
                extra.ins.sync_info = mybir.SyncInfo(
                    on_wait=waits[i : i + 1], on_update=[]
                )
        self.nc.sync.drain()
        self.nc.all_engine_barrier()
        assert self.sems is not None
        popped = self.nc._tile_sem_poison_stack.pop()
        assert popped is self._sem_poison
        self.nc.clear_and_free_semaphores(list(self.sems.allocated().values()))
        self.nc.all_engine_barrier()

    tile_mod.TileContext._drain_and_barrier = _drain_and_barrier


# ----------------------------------------------------------------------------
# host-side sharding / stream building
# ----------------------------------------------------------------------------

def _host_prep(src, dst, n_nodes, n_cores):
    """Shard edges by src range, sort by src, build per-core device streams."""
    import ml_dtypes

    NV = n_nodes // n_cores
    NW0 = (NV + P - 1) // P
    NS = (NW0 + SW - 1) // SW         # supers
    NW = NS * SW                      # padded to whole supers (tail windows empty)
    NP_ = ((n_nodes + P - 1) // P) * P
    BANK = NP_ // NB
    src = np.asarray(src)
    dst = np.asarray(dst)

    cores = []
    T_b = 1
    for c in range(n_cores):
        lo = c * NV
        sel = (src >= lo) & (src < lo + NV)
        es = src[sel].astype(np.int64) - lo
        ed = dst[sel].astype(np.int64)
        order = np.argsort(es, kind="stable")
        es = es[order]
        ed = ed[order]
        u, counts = np.unique(es, return_counts=True)
        K = len(u)
        ranks = np.repeat(np.arange(K, dtype=np.int64), counts)
        w = ranks // P
        b = ed // BANK
        # per (window, bank) counts
        cnt_wb = np.zeros((NW, NB), np.int64)
        np.add.at(cnt_wb, (w, b), 1)
        T_b = max(T_b, int(math.ceil(cnt_wb.max() / P)))
        cores.append((ed, u, K, ranks, w, b, cnt_wb))

    T_w = NB * T_b
    NCOL = NS * NB * SW * T_b

    per_core = []
    for c in range(n_cores):
        ed, u, K, ranks, w, b, cnt_wb = cores[c]
        lo = c * NV

        # order edges by (window, bank, rank-order) and compute positions
        key = (w * NB + b)
        order2 = np.argsort(key, kind="stable")
        ed2 = ed[order2]
        w2 = w[order2]
        b2 = b[order2]
        r2 = ranks[order2]
        # position within the (w, b) group
        offs = np.zeros(NW * NB + 1, np.int64)
        offs[1:] = np.cumsum(cnt_wb.reshape(-1))
        pos = np.arange(len(ed2)) - offs[key[order2]]
        t = pos // P
        p = pos % P
        s = w2 // SW
        wi = w2 % SW
        col = ((s * NB + b2) * SW + wi) * T_b + t

        rankc = np.full((P, NCOL), -1.0, np.float32)
        rankc[p, col] = (r2 % P).astype(np.float32)

        # int16 gather index stream: flat position = col*128 + p;
        # value = bank-local row of dst
        gidx = np.zeros((16, NCOL * P // 16), np.int16)
        fpos = col * P + p
        gidx[fpos % 16, fpos // 16] = (ed2 - b2 * BANK).astype(np.int16)
        gidx = np.tile(gidx, (8, 1))  # replicate across the 8 Q7 cores

        u_pad = np.zeros(NW * P, np.int32)
        u_pad[:K] = (u + lo).astype(np.int32)
        uidx = u_pad.reshape(NW, P).T.copy()  # [128, NW]

        per_core.append(
            dict(
                gidx=gidx,
                rankc=rankc.astype(ml_dtypes.bfloat16),
                uidx=uidx,
                u=u,
                K=K,
            )
        )

    cfg = dict(NV=NV, NW=NW, T_w=T_w, T_b=T_b, NS=NS)
    return cfg, per_core


# ----------------------------------------------------------------------------
# device program
# ----------------------------------------------------------------------------

def _split_multi_waits(nc):
    """This env's walrus accepts at most ONE sync-wait command per
    instruction.  Move extra waits onto single-wait nops inserted just
    before the instruction on the same engine."""
    import concourse.mybir as mybir

    cnt = 0
    for f in nc.m.functions:
        for blk in f.blocks:
            insts = blk.instructions
            out = []
            changed = False
            for ins in insts:
                si = ins.sync_info
                waits = list(si.on_wait) if si is not None else []
                if len(waits) > 1:
                    changed = True
                    for w in waits[:-1]:
                        cnt += 1
                        nop = mybir.InstNoOp(
                            name=f"wsplit_{cnt}", ins=[], outs=[]
                        )
                        nop.engine = ins.engine
                        nop.sync_info = mybir.SyncInfo(on_wait=[w], on_update=[])
                        out.append(nop)
                    si.on_wait = waits[-1:]
                    ins.sync_info = si
                out.append(ins)
            if changed:
                blk.instructions = out
    return cnt


def _build_nc(N, NW, T_w, split_waits=True):
    _apply_tile_patch()
    import concourse.bass as bass
    import concourse.mybir as mybir
    import concourse.tile as tile
    from concourse.masks import make_identity
    from concourse import library_config
    from concourse.library_overlay import lower_extended_insts

    f32 = mybir.dt.float32
    bf16 = mybir.dt.bfloat16
    i32 = mybir.dt.int32
    i16 = mybir.dt.int16
    AF = mybir.ActivationFunctionType
    OP = mybir.AluOpType
    IOOA = bass.IndirectOffsetOnAxis

    assert T_w % NB == 0
    T_b = T_w // NB
    NS = (NW + SW - 1) // SW
    assert NS * SW == NW, (NS, NW)
    NCOL = NS * NB * SW * T_b
    RUN = SW * T_b              # tiles per (super, bank) gather run
    NP_ = ((N + P - 1) // P) * P
    BANK = NP_ // NB
    ntileA = NP_ // P

    nc = bass.Bass()
    xT_d = nc.declare_dram_parameter("xT", [IN_F, NP_], bf16, isOutput=False)
    wl_d = nc.declare_dram_parameter("wl", [IN_F, IN_F], f32, isOutput=False)
    aw_d = nc.declare_dram_parameter("aw", [L, 2 * IN_F], f32, isOutput=False)
    ew_d = nc.declare_dram_parameter("ew", [IN_F, D_EM], f32, isOutput=False)
    bl_d = nc.declare_dram_parameter("bl", [IN_F, 1], f32, isOutput=False)
    embb_d = nc.declare_dram_parameter("embb", [1, D_EM], f32, isOutput=False)
    attb_d = nc.declare_dram_parameter("attb", [1, L], f32, isOutput=False)
    gidx_d = nc.declare_dram_parameter("gidx", [P, NCOL * P // 16], i16, isOutput=False)
    rankc_d = nc.declare_dram_parameter("rankc", [P, NCOL], bf16, isOutput=False)
    uidx_d = nc.declare_dram_parameter("uidx", [P, NW], i32, isOutput=False)
    iota_d = nc.declare_dram_parameter("iota_mat", [P, P], bf16, isOutput=False)
    out_d = nc.declare_dram_parameter("out", [NW * P, 4 * D_EM], f32, isOutput=True)

    tbl = nc.dram_tensor("tbl", [NP_, P], bf16)  # [h_em(64)|s_dst+attb(4)|s_src(4)|0]

    with ExitStack() as ctx:
        tc = ctx.enter_context(tile.TileContext(nc))
        nc.gpsimd.load_library(library_config.mlp)
        const = ctx.enter_context(tc.tile_pool(name="const", bufs=1))

        ident = const.tile([P, P], f32)
        make_identity(nc, ident[:])
        identb = const.tile([P, P], bf16)
        nc.vector.tensor_copy(out=identb[:], in_=ident[:])
        iotab = const.tile([P, P], bf16)
        nc.sync.dma_start(out=iotab[:], in_=iota_d[:])

        # ---- fold weights (f32): Wp[ic] = [emb_w | W@a_dst.T | W@a_src.T | 0]
        WT = [[const.tile([P, P], f32, name=f"WT_{j}_{i}") for i in range(2)] for j in range(2)]
        adT = [const.tile([P, L], f32, name=f"adT_{j}") for j in range(2)]
        asT = [const.tile([P, L], f32, name=f"asT_{j}") for j in range(2)]
        blT = [const.tile([P, 1], f32, name=f"blT_{j}") for j in range(2)]
        Wp = [const.tile([P, P], f32, name=f"Wp_{i}") for i in range(2)]
        Wpb = [const.tile([P, P], bf16, name=f"Wpb_{i}") for i in range(2)]
        bias_row = const.tile([1, P], f32)
        bias_rowb = const.tile([1, P], bf16)
        ones1b = const.tile([1, P], bf16)
        attb_sb = const.tile([1, L], f32)

        with (
            tc.tile_pool(name="setup_sb", bufs=2) as ssb,
            tc.tile_pool(name="setup_ps", bufs=2, space="PSUM") as sps,
        ):
            for jc in range(2):
                nc.sync.dma_start(
                    out=adT[jc][:],
                    in_=aw_d[:, IN_F + jc * P : IN_F + (jc + 1) * P].transpose([1, 0]),
                )
                nc.sync.dma_start(
                    out=asT[jc][:],
                    in_=aw_d[:, jc * P : (jc + 1) * P].transpose([1, 0]),
                )
                nc.sync.dma_start(out=blT[jc][:], in_=bl_d[jc * P : (jc + 1) * P, :])
                for ic in range(2):
                    wt = ssb.tile([P, P], f32)
                    nc.sync.dma_start(
                        out=wt[:],
                        in_=wl_d[ic * P : (ic + 1) * P, jc * P : (jc + 1) * P],
                    )
                    tp = sps.tile([P, P], f32, space="PSUM")
                    nc.tensor.transpose(out=tp[:], in_=wt[:], identity=ident[:])
                    nc.vector.tensor_copy(out=WT[jc][ic][:], in_=tp[:])

            for ic in range(2):
                nc.gpsimd.memset(Wp[ic][:], 0)
                nc.sync.dma_start(
                    out=Wp[ic][:, 0:D_EM], in_=ew_d[ic * P : (ic + 1) * P, :]
                )
                wd_ps = sps.tile([P, 2 * L], f32, space="PSUM")
                for t, rhs_t in ((0, adT), (1, asT)):
                    for jc in range(2):
                        nc.tensor.matmul(
                            out=wd_ps[:, t * L : (t + 1) * L],
                            lhsT=WT[jc][ic][:],
                            rhs=rhs_t[jc][:],
                            start=(jc == 0),
                            stop=(jc == 1),
                        )
                nc.vector.tensor_copy(
                    out=Wp[ic][:, D_EM : D_EM + 2 * L], in_=wd_ps[:]
                )
                nc.vector.tensor_copy(out=Wpb[ic][:], in_=Wp[ic][:])

            bias_ps = sps.tile([1, 2 * L], f32, space="PSUM")
            for t, rhs_t in ((0, adT), (1, asT)):
                for jc in range(2):
                    nc.tensor.matmul(
                        out=bias_ps[:, t * L : (t + 1) * L],
                        lhsT=blT[jc][:],
                        rhs=rhs_t[jc][:],
                        start=(jc == 0),
                        stop=(jc == 1),
                    )
            nc.gpsimd.memset(bias_row[:], 0)
            nc.sync.dma_start(out=bias_row[:, 0:D_EM], in_=embb_d[:])
            nc.sync.dma_start(out=attb_sb[:], in_=attb_d[:])
            nc.vector.tensor_tensor(
                out=bias_row[:, D_EM : D_EM + L],
                in0=bias_ps[:, 0:L],
                in1=attb_sb[:],
                op=OP.add,
            )
            nc.vector.tensor_copy(
                out=bias_row[:, D_EM + L : D_EM + 2 * L], in_=bias_ps[:, L : 2 * L]
            )
            nc.vector.tensor_copy(out=bias_rowb[:], in_=bias_row[:])
            nc.gpsimd.memset(ones1b[:], 1.0)

        # ---- phase A: tbl[N, 128] = x @ Wp + bias (bf16 matmuls, f32 out) ----
        with (
            tc.tile_pool(name="xa", bufs=2) as xap,
            tc.tile_pool(name="stgA", bufs=4) as stgA,
            tc.tile_pool(name="psM", bufs=4, space="PSUM") as psM,
        ):
            for b0 in range(0, ntileA, TB_A):
                bn = min(TB_A, ntileA - b0)
                xa0 = xap.tile([P, TB_A * P], bf16)
                xa1 = xap.tile([P, TB_A * P], bf16)
                nc.sync.dma_start(
                    out=xa0[:, : bn * P], in_=xT_d[0:P, b0 * P : (b0 + bn) * P]
                )
                nc.sync.dma_start(
                    out=xa1[:, : bn * P], in_=xT_d[P : 2 * P, b0 * P : (b0 + bn) * P]
                )
                st8 = stgA.tile([P, TB_A, P], bf16)
                for t in range(bn):
                    ps = psM.tile([P, P], f32, space="PSUM")
                    nc.tensor.matmul(
                        out=ps[:],
                        lhsT=xa0[:, t * P : (t + 1) * P],
                        rhs=Wpb[0][:],
                        start=True,
                        stop=False,
                    )
                    nc.tensor.matmul(
                        out=ps[:],
                        lhsT=xa1[:, t * P : (t + 1) * P],
                        rhs=Wpb[1][:],
                        start=False,
                        stop=False,
                    )
                    nc.tensor.matmul(
                        out=ps[:],
                        lhsT=ones1b[:],
                        rhs=bias_rowb[:],
                        start=False,
                        stop=True,
                    )
                    nc.scalar.copy(out=st8[:, t, :], in_=ps[:])
                nc.sync.dma_start(
                    out=tbl[b0 * P : (b0 + bn) * P, :].rearrange(
                        "(t p) e -> p t e", t=bn
                    ),
                    in_=st8[:, :bn, :],
                )

        # ---- upfront: per-window s_src gather + streams ----
        uix = const.tile([P, NW], i32)
        nc.sync.dma_start(out=uix[:], in_=uidx_d[:])
        ssrcb = const.tile([P, NW, L], bf16)
        for w in range(NW):
            nc.gpsimd.indirect_dma_start(
                out=ssrcb[:, w, :],
                out_offset=None,
                in_=tbl[:, :],
                in_offset=IOOA(ap=uix[:, w : w + 1], axis=0),
                element_offset=D_EM + L,
            )
        rankc_sb = const.tile([P, NCOL], bf16)
        nc.sync.dma_start(out=rankc_sb[:], in_=rankc_d[:])
        # iota replicated along the tile axis: iota_rep[p, n, t] = n
        iota_rep = const.tile([P, P, RUN], bf16)
        nc.vector.tensor_copy(
            out=iota_rep[:],
            in_=iotab[:].unsqueeze(2).to_broadcast([P, P, RUN]),
        )

        # ---- edge phase ----
        # pre-allocate gpsimd registers for gather counts (one per value)
        _qn_regs = {}
        for q0 in range(0, RUN, 8):
            qv = min(8, RUN - q0) * P
            if qv not in _qn_regs:
                _qn_regs[qv] = nc.gpsimd.to_reg(qv)
        with (
            tc.tile_pool(name="gx", bufs=2) as gxp,
            tc.tile_pool(name="g", bufs=2) as gpool,
            tc.tile_pool(name="s", bufs=2) as sp,
            tc.tile_pool(name="st", bufs=3) as stp,
            tc.tile_pool(name="z", bufs=3) as zp,
            tc.tile_pool(name="rev", bufs=3) as revp,
            tc.tile_pool(name="onorm", bufs=2) as onp,
            tc.tile_pool(name="psT", bufs=2, space="PSUM") as psT,
            tc.tile_pool(name="psS", bufs=2, space="PSUM") as psS,
            tc.tile_pool(name="psU", bufs=1, space="PSUM") as psU,
        ):
            for s in range(NS):
                gx = gxp.tile([P, NB * RUN * P // 16], i16)
                nc.sync.dma_start(
                    out=gx[:],
                    in_=gidx_d[
                        :, s * NB * RUN * P // 16 : (s + 1) * NB * RUN * P // 16
                    ],
                )
                U_s = psU.tile([P, SW, 512], f32, space="PSUM")
                for b in range(NB):
                    G = gpool.tile([P, RUN, P], bf16)
                    # HW limit: <=1024 indices per InstDMAGatherAnt
                    for q0 in range(0, RUN, 8):
                        qn = min(8, RUN - q0)
                        nc.gpsimd.dma_gather(
                            G[:, q0 : q0 + qn, :],
                            tbl[b * BANK : (b + 1) * BANK, :],
                            gx[
                                :,
                                (b * RUN + q0) * 8 : (b * RUN + q0 + qn) * 8,
                            ],
                            qn * P,
                            _qn_regs[qn * P],
                            P,
                        )
                    c0 = (s * NB + b) * RUN
                    # S2[e, n, t] = (rank(e, t) == n)  (transposed layout so
                    # the is_equal hits the DVE 2x perf mode)
                    S2 = sp.tile([P, P, RUN], bf16)
                    nc.vector.tensor_tensor(
                        out=S2[:],
                        in0=rankc_sb[:, c0 : c0 + RUN]
                        .unsqueeze(1)
                        .to_broadcast([P, P, RUN]),
                        in1=iota_rep[:],
                        op=OP.is_equal,
                    )
                    # ST via PE transpose (groups of 8 -> one PSUM bank) and
                    # per-tile se matmuls against the window's s_src rows
                    STb = stp.tile([P, RUN, P], bf16)
                    for g0 in range(0, RUN, 8):
                        gn = min(8, RUN - g0)
                        pT = psT.tile([P, 8, P], bf16, space="PSUM")
                        for j in range(gn):
                            nc.tensor.transpose(
                                out=pT[:, j, :],
                                in_=S2[:, :, g0 + j],
                                identity=identb[:],
                            )
                        nc.scalar.copy(
                            out=STb[:, g0 : g0 + gn, :], in_=pT[:, :gn, :]
                        )
                    se_ps = psS.tile([P, RUN, L], f32, space="PSUM")
                    for t in range(RUN):
                        wi = t // T_b
                        nc.tensor.matmul(
                            out=se_ps[:, t, :],
                            lhsT=STb[:, t, :],
                            rhs=ssrcb[:, s * SW + wi, :],
                            start=True,
                            stop=True,
                        )
                    zt = zp.tile([P, RUN, L], f32)
                    nc.vector.tensor_tensor(
                        out=zt[:],
                        in0=se_ps[:],
                        in1=G[:, :, D_EM : D_EM + L],
                        op=OP.add,
                    )
                    sg = zp.tile([P, RUN, L], f32)
                    nc.scalar.activation(out=sg[:], in_=zt[:], func=AF.Sigmoid)
                    ev = zp.tile([P, RUN, L], f32)
                    nc.scalar.activation(out=ev[:], in_=sg[:], func=AF.Exp)
                    ev8 = zp.tile([P, RUN, L, 8], bf16)
                    nc.vector.tensor_copy(
                        out=ev8[:],
                        in_=ev[:].unsqueeze(3).to_broadcast([P, RUN, L, 8]),
                    )
                    for k0 in range(0, RUN, SUB):
                        kn = min(SUB, RUN - k0)
                        Rev = revp.tile([P, SUB, 4 * D_EM + L], bf16)
                        for l in range(L):
                            nc.vector.tensor_tensor(
                                out=Rev[
                                    :, :kn, l * D_EM : (l + 1) * D_EM
                                ].rearrange("p t (dh dl) -> p t dh dl", dh=8),
                                in0=G[
                                    :, k0 : k0 + kn, 0:D_EM
                                ].rearrange("p t (dh dl) -> p t dh dl", dh=8),
                                in1=ev8[:, k0 : k0 + kn, l, :]
                                .unsqueeze(2)
                                .to_broadcast([P, kn, 8, 8]),
                                op=OP.mult,
                            )
                        nc.vector.tensor_copy(
                            out=Rev[:, :kn, 4 * D_EM : 4 * D_EM + L],
                            in_=ev[:, k0 : k0 + kn, :],
                        )
                        for i in range(kn):
                            t = k0 + i
                            wi = t // T_b
                            tb = t % T_b
                            nc.tensor.matmul(
                                out=U_s[:, wi, 0 : 4 * D_EM + L],
                                lhsT=S2[:, :, t],
                                rhs=Rev[:, i, :],
                                start=(b == 0 and tb == 0),
                                stop=(b == NB - 1 and tb == T_b - 1),
                            )
                for wi in range(SW):
                    w = s * SW + wi
                    dn = onp.tile([P, L], f32)
                    nc.vector.tensor_scalar(
                        out=dn[:],
                        in0=U_s[:, wi, 4 * D_EM : 4 * D_EM + L],
                        scalar1=1e-30,
                        scalar2=None,
                        op0=OP.add,
                    )
                    dnr = onp.tile([P, L], f32)
                    nc.vector.reciprocal(out=dnr[:], in_=dn[:])
                    ot = onp.tile([P, 4 * D_EM], f32)
                    nc.vector.tensor_tensor(
                        out=ot[:].rearrange("p (l d) -> p l d", l=L),
                        in0=U_s[:, wi, 0 : 4 * D_EM].rearrange(
                            "p (l d) -> p l d", l=L
                        ),
                        in1=dnr[:].unsqueeze(2).to_broadcast([P, L, D_EM]),
                        op=OP.mult,
                    )
                    nc.sync.dma_start(
                        out=out_d[w * P : (w + 1) * P, :], in_=ot[:]
                    )

    if split_waits:
        _split_multi_waits(nc)
    lower_extended_insts(nc)
    return nc


# ----------------------------------------------------------------------------
# public entry point
# ----------------------------------------------------------------------------

_NC_CACHE = {}


def _get_nc(N, NW, T_w):
    key = (N, NW, T_w)
    if key not in _NC_CACHE:
        _NC_CACHE[key] = _build_nc(N, NW, T_w)
    return _NC_CACHE[key]


def _make_in_maps(x, W_lin, b_lin, att_w, att_b, emb_w, emb_b, per_core, n_cores):
    import ml_dtypes

    x = np.asarray(x, np.float32)
    N = x.shape[0]
    NP_ = ((N + P - 1) // P) * P
    xT = np.zeros((IN_F, NP_), ml_dtypes.bfloat16)
    xT[:, :N] = x.T.astype(ml_dtypes.bfloat16)
    shared = dict(
        xT=np.ascontiguousarray(xT),
        wl=np.ascontiguousarray(np.asarray(W_lin, np.float32)),
        aw=np.ascontiguousarray(np.asarray(att_w, np.float32)),
        ew=np.ascontiguousarray(np.asarray(emb_w, np.float32)),
        bl=np.ascontiguousarray(np.asarray(b_lin, np.float32).reshape(-1, 1)),
        embb=np.ascontiguousarray(np.asarray(emb_b, np.float32).reshape(1, -1)),
        attb=np.ascontiguousarray(np.asarray(att_b, np.float32).reshape(1, -1)),
        iota_mat=np.ascontiguousarray(
            np.broadcast_to(np.arange(P), (P, P)).astype(ml_dtypes.bfloat16)
        ),
    )
    in_maps = []
    for c in range(n_cores):
        m = dict(shared)
        m["gidx"] = per_core[c]["gidx"]
        m["rankc"] = per_core[c]["rankc"]
        m["uidx"] = per_core[c]["uidx"]
        in_maps.append(m)
    return in_maps


def kernel(x, src, dst, W_lin, b_lin, att_w, att_b, emb_w, emb_b):
    from concourse.bass_utils import run_bass_kernel_spmd

    x = np.asarray(x)
    N = x.shape[0]
    cfg, per_core = _host_prep(src, dst, N, CORES)
    nc = _get_nc(N, cfg["NW"], cfg["T_w"])
    in_maps = _make_in_maps(
        x, W_lin, b_lin, att_w, att_b, emb_w, emb_b, per_core, CORES
    )
    res = run_bass_kernel_spmd(nc, in_maps, list(range(CORES)))
    out = np.zeros((N, 4 * D_EM), np.float32)
    NV = cfg["NV"]
    for c in range(CORES):
        K = per_core[c]["K"]
        u = per_core[c]["u"]
        out[c * NV + u] = res.results[c]["out"][:K]
    return out


# revision 23
# speedup vs baseline: 1.2520x; 1.0051x over previous
"""Bass/Trainium2 kernel for nn_DisentangleLayer (FactorGCN-style GNN layer).

Math (per reference):
  h    = x @ W_lin + b_lin                    [N, 256]
  h_em = x @ emb_w + emb_b                    [N, 64]
  s_src = h @ a_src.T ; s_dst = h @ a_dst.T   [N, 4]    (att_w = [a_src | a_dst])
  ev   = exp(sigmoid(s_src[src] + s_dst[dst] + att_b))  [E, 4]
  denom = segsum_src(ev)                       [N, 4]
  out[n, 64l:64l+64] = segsum_src(ev_l * h_em[dst]) / denom[n, l]

Strategy (v4):
  * Host shards edges by src-range across 8 cores (each core owns NV=12500
    nodes; no cross-core reduction).  Edges are sorted by src, mapped to
    dense ranks, grouped into 128-rank windows.  Windows are grouped into
    supers of SW=7; within each window edges are split by dst bank
    (4 banks of NP/4 table rows so bank-local indices fit int16) and each
    (window, bank) run is padded to a uniform T_b tiles of 128 edge slots.
    Slot column order is (super, bank, window-in-super, tile) so one
    InstDMAGatherAnt per (super, bank) bulk-gathers 7*T_b*128 table rows.
  * Phase A builds the packed per-node table tbl[N,128] f32 =
    [h_em(64) | s_dst+att_b(4) | s_src(4) | pad] with bf16 matmuls from a
    host-transposed bf16 x; bias via a K=1 ones matmul.
  * Edge phase: S one-hot from rankc via DVE is_equal; ST = PE transpose of
    S (batched PSUM->SBUF copies on Act); per-tile se matmuls against
    per-window gathered s_src rows; ev = exp(sigmoid(se + s_dst)) on Act;
    Rev = [ev_l*h_em | ev] on DVE; per-tile bf16 matmuls U_ps[w] += S^T@Rev
    accumulate each window's segment sums in PSUM; normalize and store.
"""

import math
import numpy as np
from contextlib import ExitStack

P = 128
CORES = 8
IN_F = 256
D_EM = 64
L = 4
SW = 4      # windows per super
NB = 4      # dst banks
SUB = 14    # tiles per elementwise sub-chunk
TB_A = 8    # node tiles per phase-A block

_PATCHED = False


def _apply_tile_patch():
    """walrus in this env rejects >1 sem-wait on one instruction; split the
    TileContext exit-drain waits across single-wait nops."""
    global _PATCHED
    if _PATCHED:
        return
    _PATCHED = True
    import concourse.tile as tile_mod
    import concourse.mybir as mybir
    from concourse.vector_clock import ScopedClock

    def _drain_and_barrier(self, tick_clock, wait_clock):
        nop = self.nc.sync.nop()
        wait_clock.add_sem_waits(nop.ins, ScopedClock({None: tick_clock.global_clock}))
        si = nop.ins.sync_info
        waits = list(si.on_wait) if si is not None else []
        if len(waits) > 1:
            si.on_wait = waits[:1]
            nop.ins.sync_info = si
            for i in range(1, len(waits)):
                extra = self.nc.sync.nop()
                extra.ins.sync_info = mybir.SyncInfo(
                    on_wait=waits[i : i + 1], on_update=[]
                )
        self.nc.sync.drain()
        self.nc.all_engine_barrier()
        assert self.sems is not None
        popped = self.nc._tile_sem_poison_stack.pop()
        assert popped is self._sem_poison
        self.nc.clear_and_free_semaphores(list(self.sems.allocated().values()))
        self.nc.all_engine_barrier()

    tile_mod.TileContext._drain_and_barrier = _drain_and_barrier


# ----------------------------------------------------------------------------
# host-side sharding / stream building
# ----------------------------------------------------------------------------

def _host_prep(src, dst, n_nodes, n_cores):
    """Shard edges by src range, sort by src, build per-core device streams."""
    import ml_dtypes

    NV = n_nodes // n_cores
    NW0 = (NV + P - 1) // P
    NS = (NW0 + SW - 1) // SW         # supers
    NW = NS * SW                      # padded to whole supers (tail windows empty)
    NP_ = ((n_nodes + P - 1) // P) * P
    BANK = NP_ // NB
    src = np.asarray(src)
    dst = np.asarray(dst)

    cores = []
    T_b = 1
    for c in range(n_cores):
        lo = c * NV
        sel = (src >= lo) & (src < lo + NV)
        es = src[sel].astype(np.int64) - lo
        ed = dst[sel].astype(np.int64)
        order = np.argsort(es, kind="stable")
        es = es[order]
        ed = ed[order]
        u, counts = np.unique(es, return_counts=True)
        K = len(u)
        ranks = np.repeat(np.arange(K, dtype=np.int64), counts)
        w = ranks // P
        b = ed // BANK
        # per (window, bank) counts
        cnt_wb = np.zeros((NW, NB), np.int64)
        np.add.at(cnt_wb, (w, b), 1)
        T_b = max(T_b, int(math.ceil(cnt_wb.max() / P)))
        cores.append((ed, u, K, ranks, w, b, cnt_wb))

    T_w = NB * T_b
    NCOL = NS * NB * SW * T_b

    per_core = []
    for c in range(n_cores):
        ed, u, K, ranks, w, b, cnt_wb = cores[c]
        lo = c * NV

        # order edges by (window, bank, rank-order) and compute positions
        key = (w * NB + b)
        order2 = np.argsort(key, kind="stable")
        ed2 = ed[order2]
        w2 = w[order2]
        b2 = b[order2]
        r2 = ranks[order2]
        # position within the (w, b) group
        offs = np.zeros(NW * NB + 1, np.int64)
        offs[1:] = np.cumsum(cnt_wb.reshape(-1))
        pos = np.arange(len(ed2)) - offs[key[order2]]
        t = pos // P
        p = pos % P
        s = w2 // SW
        wi = w2 % SW
        col = ((s * NB + b2) * SW + wi) * T_b + t

        rankc = np.full((P, NCOL), -1.0, np.float32)
        rankc[p, col] = (r2 % P).astype(np.float32)

        # int16 gather index stream: flat position = col*128 + p;
        # value = bank-local row of dst
        gidx = np.zeros((16, NCOL * P // 16), np.int16)
        fpos = col * P + p
        gidx[fpos % 16, fpos // 16] = (ed2 - b2 * BANK).astype(np.int16)
        gidx = np.tile(gidx, (8, 1))  # replicate across the 8 Q7 cores

        u_pad = np.zeros(NW * P, np.int32)
        u_pad[:K] = (u + lo).astype(np.int32)
        uidx = u_pad.reshape(NW, P).T.copy()  # [128, NW]

        per_core.append(
            dict(
                gidx=gidx,
                rankc=rankc.astype(ml_dtypes.bfloat16),
                uidx=uidx,
                u=u,
                K=K,
            )
        )

    cfg = dict(NV=NV, NW=NW, T_w=T_w, T_b=T_b, NS=NS)
    return cfg, per_core


# ----------------------------------------------------------------------------
# device program
# ----------------------------------------------------------------------------

def _split_multi_waits(nc):
    """This env's walrus accepts at most ONE sync-wait command per
    instruction.  Move extra waits onto single-wait nops inserted just
    before the instruction on the same engine."""
    import concourse.mybir as mybir

    cnt = 0
    for f in nc.m.functions:
        for blk in f.blocks:
            insts = blk.instructions
            out = []
            changed = False
            for ins in insts:
                si = ins.sync_info
                waits = list(si.on_wait) if si is not None else []
                if len(waits) > 1:
                    changed = True
                    for w in waits[:-1]:
                        cnt += 1
                        nop = mybir.InstNoOp(
                            name=f"wsplit_{cnt}", ins=[], outs=[]
                        )
                        nop.engine = ins.engine
                        nop.sync_info = mybir.SyncInfo(on_wait=[w], on_update=[])
                        out.append(nop)
                    si.on_wait = waits[-1:]
                    ins.sync_info = si
                out.append(ins)
            if changed:
                blk.instructions = out
    return cnt


def _build_nc(N, NW, T_w, split_waits=True):
    _apply_tile_patch()
    import concourse.bass as bass
    import concourse.mybir as mybir
    import concourse.tile as tile
    from concourse.masks import make_identity
    from concourse import library_config
    from concourse.library_overlay import lower_extended_insts

    f32 = mybir.dt.float32
    bf16 = mybir.dt.bfloat16
    i32 = mybir.dt.int32
    i16 = mybir.dt.int16
    AF = mybir.ActivationFunctionType
    OP = mybir.AluOpType
    IOOA = bass.IndirectOffsetOnAxis

    assert T_w % NB == 0
    T_b = T_w // NB
    NS = (NW + SW - 1) // SW
    assert NS * SW == NW, (NS, NW)
    NCOL = NS * NB * SW * T_b
    RUN = SW * T_b              # tiles per (super, bank) gather run
    NP_ = ((N + P - 1) // P) * P
    BANK = NP_ // NB
    ntileA = NP_ // P

    nc = bass.Bass()
    xT_d = nc.declare_dram_parameter("xT", [IN_F, NP_], bf16, isOutput=False)
    wl_d = nc.declare_dram_parameter("wl", [IN_F, IN_F], f32, isOutput=False)
    aw_d = nc.declare_dram_parameter("aw", [L, 2 * IN_F], f32, isOutput=False)
    ew_d = nc.declare_dram_parameter("ew", [IN_F, D_EM], f32, isOutput=False)
    bl_d = nc.declare_dram_parameter("bl", [IN_F, 1], f32, isOutput=False)
    embb_d = nc.declare_dram_parameter("embb", [1, D_EM], f32, isOutput=False)
    attb_d = nc.declare_dram_parameter("attb", [1, L], f32, isOutput=False)
    gidx_d = nc.declare_dram_parameter("gidx", [P, NCOL * P // 16], i16, isOutput=False)
    rankc_d = nc.declare_dram_parameter("rankc", [P, NCOL], bf16, isOutput=False)
    uidx_d = nc.declare_dram_parameter("uidx", [P, NW], i32, isOutput=False)
    iota_d = nc.declare_dram_parameter("iota_mat", [P, P], bf16, isOutput=False)
    out_d = nc.declare_dram_parameter("out", [NW * P, 4 * D_EM], f32, isOutput=True)

    tbl = nc.dram_tensor("tbl", [NP_, P], bf16)  # [h_em(64)|s_dst+attb(4)|s_src(4)|0]

    with ExitStack() as ctx:
        tc = ctx.enter_context(tile.TileContext(nc))
        nc.gpsimd.load_library(library_config.mlp)
        const = ctx.enter_context(tc.tile_pool(name="const", bufs=1))

        ident = const.tile([P, P], f32)
        make_identity(nc, ident[:])
        identb = const.tile([P, P], bf16)
        nc.vector.tensor_copy(out=identb[:], in_=ident[:])
        iotab = const.tile([P, P], bf16)
        nc.sync.dma_start(out=iotab[:], in_=iota_d[:])

        # ---- fold weights (f32): Wp[ic] = [emb_w | W@a_dst.T | W@a_src.T | 0]
        WT = [[const.tile([P, P], f32, name=f"WT_{j}_{i}") for i in range(2)] for j in range(2)]
        adT = [const.tile([P, L], f32, name=f"adT_{j}") for j in range(2)]
        asT = [const.tile([P, L], f32, name=f"asT_{j}") for j in range(2)]
        blT = [const.tile([P, 1], f32, name=f"blT_{j}") for j in range(2)]
        Wp = [const.tile([P, P], f32, name=f"Wp_{i}") for i in range(2)]
        Wpb = [const.tile([P, P], bf16, name=f"Wpb_{i}") for i in range(2)]
        bias_row = const.tile([1, P], f32)
        bias_rowb = const.tile([1, P], bf16)
        ones1b = const.tile([1, P], bf16)
        attb_sb = const.tile([1, L], f32)

        with (
            tc.tile_pool(name="setup_sb", bufs=2) as ssb,
            tc.tile_pool(name="setup_ps", bufs=2, space="PSUM") as sps,
        ):
            for jc in range(2):
                nc.sync.dma_start(
                    out=adT[jc][:],
                    in_=aw_d[:, IN_F + jc * P : IN_F + (jc + 1) * P].transpose([1, 0]),
                )
                nc.sync.dma_start(
                    out=asT[jc][:],
                    in_=aw_d[:, jc * P : (jc + 1) * P].transpose([1, 0]),
                )
                nc.sync.dma_start(out=blT[jc][:], in_=bl_d[jc * P : (jc + 1) * P, :])
                for ic in range(2):
                    wt = ssb.tile([P, P], f32)
                    nc.sync.dma_start(
                        out=wt[:],
                        in_=wl_d[ic * P : (ic + 1) * P, jc * P : (jc + 1) * P],
                    )
                    tp = sps.tile([P, P], f32, space="PSUM")
                    nc.tensor.transpose(out=tp[:], in_=wt[:], identity=ident[:])
                    nc.vector.tensor_copy(out=WT[jc][ic][:], in_=tp[:])

            for ic in range(2):
                nc.gpsimd.memset(Wp[ic][:], 0)
                nc.sync.dma_start(
                    out=Wp[ic][:, 0:D_EM], in_=ew_d[ic * P : (ic + 1) * P, :]
                )
                wd_ps = sps.tile([P, 2 * L], f32, space="PSUM")
                for t, rhs_t in ((0, adT), (1, asT)):
                    for jc in range(2):
                        nc.tensor.matmul(
                            out=wd_ps[:, t * L : (t + 1) * L],
                            lhsT=WT[jc][ic][:],
                            rhs=rhs_t[jc][:],
                            start=(jc == 0),
                            stop=(jc == 1),
                        )
                nc.vector.tensor_copy(
                    out=Wp[ic][:, D_EM : D_EM + 2 * L], in_=wd_ps[:]
                )
                nc.vector.tensor_copy(out=Wpb[ic][:], in_=Wp[ic][:])

            bias_ps = sps.tile([1, 2 * L], f32, space="PSUM")
            for t, rhs_t in ((0, adT), (1, asT)):
                for jc in range(2):
                    nc.tensor.matmul(
                        out=bias_ps[:, t * L : (t + 1) * L],
                        lhsT=blT[jc][:],
                        rhs=rhs_t[jc][:],
                        start=(jc == 0),
                        stop=(jc == 1),
                    )
            nc.gpsimd.memset(bias_row[:], 0)
            nc.sync.dma_start(out=bias_row[:, 0:D_EM], in_=embb_d[:])
            nc.sync.dma_start(out=attb_sb[:], in_=attb_d[:])
            nc.vector.tensor_tensor(
                out=bias_row[:, D_EM : D_EM + L],
                in0=bias_ps[:, 0:L],
                in1=attb_sb[:],
                op=OP.add,
            )
            nc.vector.tensor_copy(
                out=bias_row[:, D_EM + L : D_EM + 2 * L], in_=bias_ps[:, L : 2 * L]
            )
            nc.vector.tensor_copy(out=bias_rowb[:], in_=bias_row[:])
            nc.gpsimd.memset(ones1b[:], 1.0)

        # ---- phase A: tbl[N, 128] = x @ Wp + bias (bf16 matmuls, f32 out) ----
        with (
            tc.tile_pool(name="xa", bufs=2) as xap,
            tc.tile_pool(name="stgA", bufs=4) as stgA,
            tc.tile_pool(name="psM", bufs=4, space="PSUM") as psM,
        ):
            for b0 in range(0, ntileA, TB_A):
                bn = min(TB_A, ntileA - b0)
                xa0 = xap.tile([P, TB_A * P], bf16)
                xa1 = xap.tile([P, TB_A * P], bf16)
                nc.sync.dma_start(
                    out=xa0[:, : bn * P], in_=xT_d[0:P, b0 * P : (b0 + bn) * P]
                )
                nc.sync.dma_start(
                    out=xa1[:, : bn * P], in_=xT_d[P : 2 * P, b0 * P : (b0 + bn) * P]
                )
                st8 = stgA.tile([P, TB_A, P], bf16)
                for q in range(0, bn, 4):
                    qb = min(4, bn - q)
                    ps4 = psM.tile([P, 4, P], f32, space="PSUM")
                    for t in range(q, q + qb):
                        nc.tensor.matmul(
                            out=ps4[:, t - q, :],
                            lhsT=xa0[:, t * P : (t + 1) * P],
                            rhs=Wpb[0][:],
                            start=True,
                            stop=False,
                        )
                        nc.tensor.matmul(
                            out=ps4[:, t - q, :],
                            lhsT=xa1[:, t * P : (t + 1) * P],
                            rhs=Wpb[1][:],
                            start=False,
                            stop=False,
                        )
                        nc.tensor.matmul(
                            out=ps4[:, t - q, :],
                            lhsT=ones1b[:],
                            rhs=bias_rowb[:],
                            start=False,
                            stop=True,
                        )
                    nc.scalar.copy(out=st8[:, q : q + qb, :], in_=ps4[:, :qb, :])
                nc.sync.dma_start(
                    out=tbl[b0 * P : (b0 + bn) * P, :].rearrange(
                        "(t p) e -> p t e", t=bn
                    ),
                    in_=st8[:, :bn, :],
                )

        # ---- upfront: per-window s_src gather + streams ----
        uix = const.tile([P, NW], i32)
        nc.sync.dma_start(out=uix[:], in_=uidx_d[:])
        ssrcb = const.tile([P, NW, L], bf16)
        for w in range(NW):
            nc.gpsimd.indirect_dma_start(
                out=ssrcb[:, w, :],
                out_offset=None,
                in_=tbl[:, :],
                in_offset=IOOA(ap=uix[:, w : w + 1], axis=0),
                element_offset=D_EM + L,
            )
        rankc_sb = const.tile([P, NCOL], bf16)
        nc.sync.dma_start(out=rankc_sb[:], in_=rankc_d[:])
        # iota replicated along the tile axis: iota_rep[p, n, t] = n
        iota_rep = const.tile([P, P, RUN], bf16)
        nc.vector.tensor_copy(
            out=iota_rep[:],
            in_=iotab[:].unsqueeze(2).to_broadcast([P, P, RUN]),
        )

        # ---- edge phase ----
        # pre-allocate gpsimd registers for gather counts (one per value)
        _qn_regs = {}
        for q0 in range(0, RUN, 8):
            qv = min(8, RUN - q0) * P
            if qv not in _qn_regs:
                _qn_regs[qv] = nc.gpsimd.to_reg(qv)
        with (
            tc.tile_pool(name="gx", bufs=3) as gxp,
            tc.tile_pool(name="g", bufs=3) as gpool,
            tc.tile_pool(name="s", bufs=3) as sp,
            tc.tile_pool(name="st", bufs=2) as stp,
            tc.tile_pool(name="z", bufs=4) as zp,
            tc.tile_pool(name="rev", bufs=3) as revp,
            tc.tile_pool(name="onorm", bufs=2) as onp,
            tc.tile_pool(name="psT", bufs=2, space="PSUM") as psT,
            tc.tile_pool(name="psS", bufs=2, space="PSUM") as psS,
            tc.tile_pool(name="psU", bufs=1, space="PSUM") as psU,
        ):
            for s in range(NS):
                gx = gxp.tile([P, NB * RUN * P // 16], i16)
                nc.sync.dma_start(
                    out=gx[:],
                    in_=gidx_d[
                        :, s * NB * RUN * P // 16 : (s + 1) * NB * RUN * P // 16
                    ],
                )
                U_s = psU.tile([P, SW, 512], f32, space="PSUM")
                for b in range(NB):
                    G = gpool.tile([P, RUN, P], bf16)
                    # HW limit: <=1024 indices per InstDMAGatherAnt
                    for q0 in range(0, RUN, 8):
                        qn = min(8, RUN - q0)
                        nc.gpsimd.dma_gather(
                            G[:, q0 : q0 + qn, :],
                            tbl[b * BANK : (b + 1) * BANK, :],
                            gx[
                                :,
                                (b * RUN + q0) * 8 : (b * RUN + q0 + qn) * 8,
                            ],
                            qn * P,
                            _qn_regs[qn * P],
                            P,
                        )
                    c0 = (s * NB + b) * RUN
                    # S2[e, n, t] = (rank(e, t) == n)  (transposed layout so
                    # the is_equal hits the DVE 2x perf mode)
                    S2 = sp.tile([P, P, RUN], bf16)
                    nc.vector.tensor_tensor(
                        out=S2[:],
                        in0=rankc_sb[:, c0 : c0 + RUN]
                        .unsqueeze(1)
                        .to_broadcast([P, P, RUN]),
                        in1=iota_rep[:],
                        op=OP.is_equal,
                    )
                    # ST via PE transpose (groups of 8 -> one PSUM bank) and
                    # per-tile se matmuls against the window's s_src rows
                    STb = stp.tile([P, RUN, P], bf16)
                    for g0 in range(0, RUN, 8):
                        gn = min(8, RUN - g0)
                        pT = psT.tile([P, 8, P], bf16, space="PSUM")
                        for j in range(gn):
                            nc.tensor.transpose(
                                out=pT[:, j, :],
                                in_=S2[:, :, g0 + j],
                                identity=identb[:],
                            )
                        nc.scalar.copy(
                            out=STb[:, g0 : g0 + gn, :], in_=pT[:, :gn, :]
                        )
                    se_ps = psS.tile([P, RUN, L], f32, space="PSUM")
                    for t in range(RUN):
                        wi = t // T_b
                        nc.tensor.matmul(
                            out=se_ps[:, t, :],
                            lhsT=STb[:, t, :],
                            rhs=ssrcb[:, s * SW + wi, :],
                            start=True,
                            stop=True,
                        )
                    zt = zp.tile([P, RUN, L], f32)
                    nc.vector.tensor_tensor(
                        out=zt[:],
                        in0=se_ps[:],
                        in1=G[:, :, D_EM : D_EM + L],
                        op=OP.add,
                    )
                    sg = zp.tile([P, RUN, L], f32)
                    nc.scalar.activation(out=sg[:], in_=zt[:], func=AF.Sigmoid)
                    Rev = revp.tile([P, RUN, 4 * D_EM + L], bf16)
                    nc.scalar.activation(
                        out=Rev[:, :, 4 * D_EM : 4 * D_EM + L],
                        in_=sg[:],
                        func=AF.Exp,
                    )
                    ev8 = zp.tile([P, RUN, L, 8], bf16)
                    nc.vector.tensor_copy(
                        out=ev8[:],
                        in_=Rev[:, :, 4 * D_EM : 4 * D_EM + L]
                        .unsqueeze(3)
                        .to_broadcast([P, RUN, L, 8]),
                    )
                    half = (RUN + 1) // 2
                    for k0 in range(0, RUN, half):
                        kn = min(half, RUN - k0)
                        for l in range(L):
                            nc.vector.tensor_tensor(
                                out=Rev[
                                    :, k0 : k0 + kn, l * D_EM : (l + 1) * D_EM
                                ].rearrange("p t (dh dl) -> p t dh dl", dh=8),
                                in0=G[:, k0 : k0 + kn, 0:D_EM].rearrange(
                                    "p t (dh dl) -> p t dh dl", dh=8
                                ),
                                in1=ev8[:, k0 : k0 + kn, l, :]
                                .unsqueeze(2)
                                .to_broadcast([P, kn, 8, 8]),
                                op=OP.mult,
                            )
                        for t in range(k0, k0 + kn):
                            wi = t // T_b
                            tb = t % T_b
                            nc.tensor.matmul(
                                out=U_s[:, wi, 0 : 4 * D_EM + L],
                                lhsT=S2[:, :, t],
                                rhs=Rev[:, t, :],
                                start=(b == 0 and tb == 0),
                                stop=(b == NB - 1 and tb == T_b - 1),
                            )
                for wi in range(SW):
                    w = s * SW + wi
                    dn = onp.tile([P, L], f32)
                    nc.vector.tensor_scalar(
                        out=dn[:],
                        in0=U_s[:, wi, 4 * D_EM : 4 * D_EM + L],
                        scalar1=1e-30,
                        scalar2=None,
                        op0=OP.add,
                    )
                    dnr = onp.tile([P, L], f32)
                    nc.vector.reciprocal(out=dnr[:], in_=dn[:])
                    ot = onp.tile([P, 4 * D_EM], f32)
                    nc.vector.tensor_tensor(
                        out=ot[:].rearrange("p (l d) -> p l d", l=L),
                        in0=U_s[:, wi, 0 : 4 * D_EM].rearrange(
                            "p (l d) -> p l d", l=L
                        ),
                        in1=dnr[:].unsqueeze(2).to_broadcast([P, L, D_EM]),
                        op=OP.mult,
                    )
                    nc.sync.dma_start(
                        out=out_d[w * P : (w + 1) * P, :], in_=ot[:]
                    )

    if split_waits:
        _split_multi_waits(nc)
    lower_extended_insts(nc)
    return nc


# ----------------------------------------------------------------------------
# public entry point
# ----------------------------------------------------------------------------

_NC_CACHE = {}


def _get_nc(N, NW, T_w):
    key = (N, NW, T_w)
    if key not in _NC_CACHE:
        _NC_CACHE[key] = _build_nc(N, NW, T_w)
    return _NC_CACHE[key]


def _make_in_maps(x, W_lin, b_lin, att_w, att_b, emb_w, emb_b, per_core, n_cores):
    import ml_dtypes

    x = np.asarray(x, np.float32)
    N = x.shape[0]
    NP_ = ((N + P - 1) // P) * P
    xT = np.zeros((IN_F, NP_), ml_dtypes.bfloat16)
    xT[:, :N] = x.T.astype(ml_dtypes.bfloat16)
    shared = dict(
        xT=np.ascontiguousarray(xT),
        wl=np.ascontiguousarray(np.asarray(W_lin, np.float32)),
        aw=np.ascontiguousarray(np.asarray(att_w, np.float32)),
        ew=np.ascontiguousarray(np.asarray(emb_w, np.float32)),
        bl=np.ascontiguousarray(np.asarray(b_lin, np.float32).reshape(-1, 1)),
        embb=np.ascontiguousarray(np.asarray(emb_b, np.float32).reshape(1, -1)),
        attb=np.ascontiguousarray(np.asarray(att_b, np.float32).reshape(1, -1)),
        iota_mat=np.ascontiguousarray(
            np.broadcast_to(np.arange(P), (P, P)).astype(ml_dtypes.bfloat16)
        ),
    )
    in_maps = []
    for c in range(n_cores):
        m = dict(shared)
        m["gidx"] = per_core[c]["gidx"]
        m["rankc"] = per_core[c]["rankc"]
        m["uidx"] = per_core[c]["uidx"]
        in_maps.append(m)
    return in_maps


def kernel(x, src, dst, W_lin, b_lin, att_w, att_b, emb_w, emb_b):
    from concourse.bass_utils import run_bass_kernel_spmd

    x = np.asarray(x)
    N = x.shape[0]
    cfg, per_core = _host_prep(src, dst, N, CORES)
    nc = _get_nc(N, cfg["NW"], cfg["T_w"])
    in_maps = _make_in_maps(
        x, W_lin, b_lin, att_w, att_b, emb_w, emb_b, per_core, CORES
    )
    res = run_bass_kernel_spmd(nc, in_maps, list(range(CORES)))
    out = np.zeros((N, 4 * D_EM), np.float32)
    NV = cfg["NV"]
    for c in range(CORES):
        K = per_core[c]["K"]
        u = per_core[c]["u"]
        out[c * NV + u] = res.results[c]["out"][:K]
    return out
